# revision 1
# baseline (speedup 1.0000x reference)
"""BinaryGPTNeoBlock on 8 trn2 NeuronCores.

Sequence-parallel over 8 cores: core c owns rows {c, c+8, ...} of both
batch elements (256 per batch, 512 total). One 8-core AllGather shares
K/V in bf16 (feature-major K, token-major V); two more share tanh'd MLP
weights (each core tanh's 1/8th). QKV/out-proj/fc matmuls run fp32r
(full PE rate at N>=256); attention and the FF->D projection run bf16.

Self-contained: hardcodes shapes; host only shards/transposes/builds masks.
"""

import numpy as np
import ml_dtypes

import concourse.bass as bass
import concourse.tile as tile
from concourse import bacc, mybir
from concourse.bass_utils import run_bass_kernel_spmd
from concourse.masks import make_identity

B, S, D = 2, 2048, 2048
H = 16
HD = 128
FF = 4 * D
EPS = 1e-5
NC = 8
RPC = S // NC          # 256 rows per core per batch
TL = 2 * RPC           # 512 local rows
NKV = TL * D           # elems of K^T (== of V) per core
WFC_CH = D * FF // NC
WPJ_CH = FF * D // NC

dt = mybir.dt
AF = mybir.ActivationFunctionType
OP = mybir.AluOpType

_CACHE = {}


def _build(apply_g1, apply_b1, apply_g2, apply_b2):
    nc = bacc.Bacc("TRN2", target_bir_lowering=False, debug=False,
                   num_devices=NC)

    xl_d = nc.dram_tensor("xl", [TL, D], dt.float32, kind="ExternalInput").ap()
    wqT_d = nc.dram_tensor("wqT", [D, D], dt.float32, kind="ExternalInput").ap()
    wkT_d = nc.dram_tensor("wkT", [D, D], dt.float32, kind="ExternalInput").ap()
    wvT_d = nc.dram_tensor("wvT", [D, D], dt.float32, kind="ExternalInput").ap()
    woT_d = nc.dram_tensor("woT", [D, D], dt.float32, kind="ExternalInput").ap()
    wfc_ch_d = nc.dram_tensor("wfc_ch", [WFC_CH], dt.float32,
                              kind="ExternalInput").ap()
    wpj_ch_d = nc.dram_tensor("wpj_ch", [WPJ_CH], dt.float32,
                              kind="ExternalInput").ap()
    mask_d = nc.dram_tensor("mask", [128, 4, 2, 512], dt.bfloat16,
                            kind="ExternalInput").ap()
    ln1g_d = nc.dram_tensor("ln1g", [D], dt.float32, kind="ExternalInput").ap()
    ln1b_d = nc.dram_tensor("ln1b", [D], dt.float32, kind="ExternalInput").ap()
    ln2g_d = nc.dram_tensor("ln2g", [D], dt.float32, kind="ExternalInput").ap()
    ln2b_d = nc.dram_tensor("ln2b", [D], dt.float32, kind="ExternalInput").ap()
    bo_d = nc.dram_tensor("bo", [D], dt.float32, kind="ExternalInput").ap()
    bfc_d = nc.dram_tensor("bfc", [FF], dt.float32, kind="ExternalInput").ap()
    bpj_d = nc.dram_tensor("bpj", [D], dt.float32, kind="ExternalInput").ap()
    out_d = nc.dram_tensor("out", [TL, D], dt.float32,
                           kind="ExternalOutput").ap()

    def bcast_row(src_ap, n):
        return bass.AP(tensor=src_ap.tensor, offset=src_ap.offset,
                       ap=[[0, 128], [1, n]])

    with tile.TileContext(nc) as tc:
        import contextlib
        stack = contextlib.ExitStack()
        main = stack.enter_context(tc.tile_pool(name="main", bufs=1))
        dram = stack.enter_context(
            tc.tile_pool(name="dram", bufs=1, space="DRAM"))

        ident = main.tile([128, 128], dt.float32)
        make_identity(nc, ident[:])
        ones_col = main.tile([128, 1], dt.float32)
        nc.vector.memset(ones_col[:], 1.0)
        ones_col_b = main.tile([128, 1], dt.bfloat16)
        nc.vector.tensor_copy(ones_col_b[:], ones_col[:])
        ones_row = main.tile([1, 128], dt.float32)
        nc.vector.memset(ones_row[:], 1.0)
        eps_t = main.tile([128, 1], dt.float32)
        nc.vector.memset(eps_t[:], EPS)
        bo_bc = main.tile([128, D], dt.float32)
        nc.sync.dma_start(out=bo_bc[:], in_=bcast_row(bo_d, D))
        bpj_bc = main.tile([128, D], dt.float32)
        nc.sync.dma_start(out=bpj_bc[:], in_=bcast_row(bpj_d, D))
        ln_bc = {}
        for nm, flag, src in (("g1", apply_g1, ln1g_d),
                              ("b1", apply_b1, ln1b_d),
                              ("g2", apply_g2, ln2g_d),
                              ("b2", apply_b2, ln2b_d)):
            if flag:
                t = main.tile([128, D], dt.float32, name=f"ln_{nm}")
                nc.sync.dma_start(out=t[:], in_=bcast_row(src, D))
                ln_bc[nm] = t
        bfc_pp = main.tile([128, FF // 128], dt.float32)
        nc.sync.dma_start(
            out=bfc_pp[:],
            in_=bass.AP(tensor=bfc_d.tensor, offset=bfc_d.offset,
                        ap=[[1, 128], [128, FF // 128]]))
        mask_pool = tc.tile_pool(name="maskp", bufs=1)
        maskp = mask_pool.__enter__()
        masks = maskp.tile([128, 4, 2, 512], dt.bfloat16)
        nc.sync.dma_start(out=masks[:], in_=mask_d[:])

        # big rotating bf16 slots: hT -> OT reuse, QT -> mT reuse
        hT = main.tile([128, 16, 512], dt.bfloat16, tag="bigA", bufs=2,
                       name="hT")
        QT = main.tile([128, 16, 512], dt.bfloat16, tag="bigA", bufs=2,
                       name="QT")

        h2_d = dram.tile([TL, D], dt.float32)

        def layernorm(x_t, h_t, gk, bk):
            with tc.tile_pool(name="lnp", bufs=2) as lp:
                st = lp.tile([128, 4, 6], dt.float32, tag="st")
                xr = x_t[:].rearrange("p (n f) -> p n f", n=4)
                for sg in range(4):
                    nc.vector.bn_stats(out=st[:, sg, :], in_=xr[:, sg, :])
                mv = lp.tile([128, 2], dt.float32, tag="mv")
                nc.vector.bn_aggr(out=mv[:], in_=st[:])
                std = lp.tile([128, 1], dt.float32, tag="sd")
                nc.scalar.activation(std[:], mv[:, 1:2], AF.Sqrt,
                                     bias=eps_t[:])
                rstd = lp.tile([128, 1], dt.float32, tag="rs")
                nc.vector.reciprocal(rstd[:], std[:])
                nc.vector.tensor_scalar(h_t[:], x_t[:], mv[:, 0:1], rstd[:],
                                        op0=OP.subtract, op1=OP.mult)
                if gk in ln_bc:
                    nc.vector.tensor_mul(h_t[:], h_t[:], ln_bc[gk][:])
                if bk in ln_bc:
                    nc.vector.tensor_add(h_t[:], h_t[:], ln_bc[bk][:])

        # ---------- Phase A: x -> LN1 -> h^T ----------
        with tc.tile_pool(name="xa", bufs=2) as xa, \
             tc.tile_pool(name="ha", bufs=2) as ha, \
             tc.tile_pool(name="trps", bufs=4, space="PSUM") as trps:
            for tb in range(4):
                x_t = xa.tile([128, D], dt.float32, tag="x")
                nc.sync.dma_start(out=x_t[:],
                                  in_=xl_d[tb * 128:(tb + 1) * 128, :])
                h_t = ha.tile([128, D], dt.float32, tag="h")
                layernorm(x_t, h_t, "g1", "b1")
                for dj in range(16):
                    ps = trps.tile([128, 128], dt.float32, tag="tp")
                    nc.tensor.transpose(ps[:], h_t[:, dj * 128:(dj + 1) * 128],
                                        ident[:])
                    nc.vector.tensor_copy(hT[:, dj, tb * 128:(tb + 1) * 128],
                                          ps[:])

        # ---------- Phase B: QKV ----------
        k_bounce = dram.tile([NKV], dt.bfloat16)
        v_bounce = dram.tile([NKV], dt.bfloat16)
        k_gath = dram.tile([NC * NKV], dt.bfloat16, addr_space="Shared")
        v_gath = dram.tile([NC * NKV], dt.bfloat16, addr_space="Shared")

        def project_qk(wT_dram, kind):
            # feature-major output via PE transpose; og(4) x [128,512] loads
            with tc.tile_pool(name=f"pw_{kind}", bufs=4) as wp, \
                 tc.tile_pool(name=f"po_{kind}", bufs=4) as op_, \
                 tc.tile_pool(name=f"pp_{kind}", bufs=1, space="PSUM") as pp, \
                 tc.tile_pool(name=f"pt_{kind}", bufs=4, space="PSUM") as tp2:
                for og in range(4):
                    o_base = og * 512
                    ktacc = []
                    if kind == "k":
                        for k4 in range(4):
                            ka = op_.tile([128, 512], dt.bfloat16, tag="ka",
                                          bufs=8, name=f"ka_{og}_{k4}")
                            ktacc.append(ka)
                    pss = [None] * 4
                    for dj in range(16):
                        raw = wp.tile([128, 512], dt.float32, tag="raw")
                        nc.sync.dma_start(
                            out=raw[:],
                            in_=wT_dram[dj * 128:(dj + 1) * 128,
                                        o_base:o_base + 512])
                        tnh = wp.tile([128, 512], dt.bfloat16, tag="tnh")
                        nc.scalar.activation(tnh[:], raw[:], AF.Tanh)
                        for tb in range(4):
                            if pss[tb] is None:
                                pss[tb] = pp.tile([128, 512], dt.float32,
                                                  tag=f"ps{tb}",
                                                  name=f"ps_{kind}_{tb}")
                            nc.tensor.matmul(
                                pss[tb][:],
                                hT[:, dj, tb * 128:(tb + 1) * 128],
                                tnh[:], start=(dj == 0), stop=(dj == 15))
                    for tb in range(4):
                        tm = op_.tile([128, 512], dt.float32, tag="tm")
                        nc.scalar.activation(tm[:], pss[tb][:], AF.Copy)
                        for k4 in range(4):
                            dj2 = (o_base + k4 * 128) // 128
                            ps2 = tp2.tile([128, 128], dt.float32, tag="t2")
                            nc.tensor.transpose(
                                ps2[:], tm[:, k4 * 128:(k4 + 1) * 128],
                                ident[:])
                            if kind == "q":
                                nc.vector.tensor_copy(
                                    QT[:, dj2, tb * 128:(tb + 1) * 128],
                                    ps2[:])
                            else:
                                nc.vector.tensor_copy(
                                    ktacc[k4][:, tb * 128:(tb + 1) * 128],
                                    ps2[:])
                    if kind == "k":
                        for k4 in range(4):
                            dj2 = (o_base + k4 * 128) // 128
                            nc.sync.dma_start(
                                out=k_bounce[dj2 * 128 * TL:
                                             (dj2 + 1) * 128 * TL]
                                .rearrange("(p t) -> p t", p=128),
                                in_=ktacc[k4][:])

        def project_v(wT_dram):
            # token-major; og2(2) x [128,1024] loads; full-row stores
            with tc.tile_pool(name="pw_v", bufs=4) as wp, \
                 tc.tile_pool(name="po_v", bufs=4) as op_, \
                 tc.tile_pool(name="pp_v", bufs=1, space="PSUM") as pp:
                vacc = [op_.tile([128, D], dt.bfloat16, tag="va", bufs=4,
                                 name=f"va_{t}") for t in range(4)]
                for og2 in range(2):
                    o_base = og2 * 1024
                    pss = [None] * 8
                    for dj in range(16):
                        raw = wp.tile([128, 1024], dt.float32, tag="raw")
                        nc.sync.dma_start(
                            out=raw[:],
                            in_=wT_dram[dj * 128:(dj + 1) * 128,
                                        o_base:o_base + 1024])
                        tnh = wp.tile([128, 1024], dt.bfloat16, tag="tnh")
                        nc.scalar.activation(tnh[:], raw[:], AF.Tanh)
                        for osub in range(2):
                            for tb in range(4):
                                k = osub * 4 + tb
                                if pss[k] is None:
                                    pss[k] = pp.tile([128, 512], dt.float32,
                                                     tag=f"ps{k}",
                                                     name=f"ps_v_{k}")
                                nc.tensor.matmul(
                                    pss[k][:],
                                    hT[:, dj, tb * 128:(tb + 1) * 128],
                                    tnh[:, osub * 512:(osub + 1) * 512],
                                    start=(dj == 0), stop=(dj == 15))
                    for osub in range(2):
                        for tb in range(4):
                            sl = slice(o_base + osub * 512,
                                       o_base + osub * 512 + 512)
                            nc.scalar.activation(vacc[tb][:, sl],
                                                 pss[osub * 4 + tb][:],
                                                 AF.Copy)
                for tb in range(4):
                    nc.sync.dma_start(
                        out=v_bounce[tb * 128 * D:(tb + 1) * 128 * D]
                        .rearrange("(p t) -> p t", p=128),
                        in_=vacc[tb][:])

        project_qk(wkT_d, "k")
        nc.gpsimd.collective_compute(
            "AllGather", OP.bypass, replica_groups=[list(range(NC))],
            ins=[k_bounce[:]], outs=[k_gath[:]])
        project_v(wvT_d)
        nc.gpsimd.collective_compute(
            "AllGather", OP.bypass, replica_groups=[list(range(NC))],
            ins=[v_bounce[:]], outs=[v_gath[:]])
        project_qk(wqT_d, "q")

        # ---------- MLP weight tanh (own 1/8th) + AllGathers ----------
        wfc_bounce = dram.tile([WFC_CH], dt.bfloat16)
        wpj_bounce = dram.tile([WPJ_CH], dt.bfloat16)
        wfc_gath = dram.tile([NC * WFC_CH], dt.bfloat16, addr_space="Shared")
        wpj_gath = dram.tile([NC * WPJ_CH], dt.bfloat16, addr_space="Shared")
        with tc.tile_pool(name="wprep", bufs=3) as wprep:
            for src, dst, odt, n_t, otag in (
                    (wfc_ch_d, wfc_bounce, dt.bfloat16,
                     WFC_CH // (128 * 2048), "f"),
                    (wpj_ch_d, wpj_bounce, dt.bfloat16,
                     WPJ_CH // (128 * 2048), "p")):
                for i in range(n_t):
                    raw = wprep.tile([128, 2048], dt.float32, tag="wraw")
                    nc.sync.dma_start(
                        out=raw[:],
                        in_=src[i * 128 * 2048:(i + 1) * 128 * 2048]
                        .rearrange("(p f) -> p f", p=128))
                    tnh = wprep.tile([128, 2048], odt, tag=f"wtnh{otag}")
                    nc.scalar.activation(tnh[:], raw[:], AF.Tanh)
                    nc.sync.dma_start(
                        out=dst[i * 128 * 2048:(i + 1) * 128 * 2048]
                        .rearrange("(p f) -> p f", p=128), in_=tnh[:])
        nc.gpsimd.collective_compute(
            "AllGather", OP.bypass, replica_groups=[list(range(NC))],
            ins=[wfc_bounce[:]], outs=[wfc_gath[:]])
        nc.gpsimd.collective_compute(
            "AllGather", OP.bypass, replica_groups=[list(range(NC))],
            ins=[wpj_bounce[:]], outs=[wpj_gath[:]])
        wfcT_v = wfc_gath[:].rearrange("(d f) -> d f", d=D)    # [D, FF]
        wpjT_v = wpj_gath[:].rearrange("(f o) -> f o", f=FF)   # [FF, D]


        # ---------- Phase C: attention (bf16) ----------
        OT = main.tile([128, 16, 512], dt.bfloat16, tag="bigA", bufs=2,
                       name="OT")
        with tc.tile_pool(name="kvh", bufs=3) as kvh, \
             tc.tile_pool(name="att", bufs=4) as att, \
             tc.tile_pool(name="attsm", bufs=6) as attsm, \
             tc.tile_pool(name="stps", bufs=3, space="PSUM") as stps, \
             tc.tile_pool(name="otps", bufs=2, space="PSUM") as otps, \
             tc.tile_pool(name="dnps", bufs=2, space="PSUM") as dnps, \
             tc.tile_pool(name="bcps", bufs=1, space="PSUM") as bcps:
            for hg in range(4):            # head groups of 4
                kt_g, v_g = [], []
                for j in range(NC):
                    kt = kvh.tile([128, 4, 512], dt.bfloat16, tag="kth",
                                  bufs=12, name=f"kt_{hg}_{j}")
                    nc.sync.dma_start(
                        out=kt[:],
                        in_=bass.AP(tensor=k_gath.tensor,
                                    offset=k_gath.offset + j * NKV
                                    + hg * 4 * 128 * TL,
                                    ap=[[TL, 128], [128 * TL, 4], [1, TL]]))
                    kt_g.append(kt)
                    vt = kvh.tile([128, 4, 512], dt.bfloat16, tag="vth",
                                  bufs=12, name=f"vt_{hg}_{j}")
                    nc.sync.dma_start(
                        out=vt[:],
                        in_=bass.AP(tensor=v_gath.tensor,
                                    offset=v_gath.offset + j * NKV
                                    + hg * 4 * 128,
                                    ap=[[D, 128], [128 * D, 4], [1, 512]]))
                    v_g.append(vt)
                for hh in range(4):
                    h = hg * 4 + hh
                    for b in range(2):
                        ot_ps = otps.tile([128, 256], dt.float32, tag="ot")
                        dn_ps = dnps.tile([1, 256], dt.float32, tag="dn")
                        n_acc = 0
                        for tb in range(2):
                            for jp in range(4):
                                st = stps.tile([128, 512], dt.float32,
                                               tag="st")
                                for half in range(2):
                                    j = 2 * jp + half
                                    nc.tensor.matmul(
                                        st[:, half * 256:(half + 1) * 256],
                                        kt_g[j][:, hh,
                                                b * 256 + tb * 128:
                                                b * 256 + tb * 128 + 128],
                                        QT[:, h, b * 256:(b + 1) * 256],
                                        start=True, stop=True)
                                pt_pre = attsm.tile([128, 512], dt.bfloat16,
                                                    tag="ptp")
                                nc.vector.tensor_add(pt_pre[:], st[:],
                                                     masks[:, jp, tb, :])
                                pt = attsm.tile([128, 512], dt.bfloat16,
                                                tag="pt")
                                nc.scalar.activation(pt[:], pt_pre[:], AF.Exp)
                                for half in range(2):
                                    j = 2 * jp + half
                                    last = (tb == 1 and jp == 3 and half == 1)
                                    nc.tensor.matmul(
                                        ot_ps[:],
                                        v_g[j][:, 2 * b + tb,
                                               hh * 128:(hh + 1) * 128],
                                        pt[:, half * 256:(half + 1) * 256],
                                        start=(n_acc == 0), stop=last,
                                        skip_group_check=True)
                                    nc.tensor.matmul(
                                        dn_ps[:], ones_col_b[:],
                                        pt[:, half * 256:(half + 1) * 256],
                                        start=(n_acc == 0), stop=last,
                                        skip_group_check=True)
                                    n_acc += 1
                        dn_sb = att.tile([1, 256], dt.float32, tag="dns")
                        nc.vector.reciprocal(dn_sb[:], dn_ps[:])
                        bc_ps = bcps.tile([128, 256], dt.float32, tag="bc")
                        nc.tensor.matmul(bc_ps[:], ones_row[:], dn_sb[:],
                                         start=True, stop=True)
                        bc_sb = att.tile([128, 256], dt.float32, tag="bcs")
                        nc.vector.tensor_copy(bc_sb[:], bc_ps[:])
                        nc.vector.tensor_mul(OT[:, h, b * 256:(b + 1) * 256],
                                             ot_ps[:], bc_sb[:])

        mask_pool.__exit__(None, None, None)

        # ---------- Phase D: out-proj + residual + LN2 -> m^T ----------
        mT = main.tile([128, 16, 512], dt.bfloat16, tag="bigA", bufs=2,
                       name="mT")
        h2_pool = tc.tile_pool(name="h2a", bufs=4)
        h2a = h2_pool.__enter__()
        h2acc = [h2a.tile([128, D], dt.float32, tag="h2", bufs=4,
                          name=f"h2_{t}") for t in range(4)]
        with tc.tile_pool(name="wo", bufs=3) as wop, \
             tc.tile_pool(name="zps", bufs=1, space="PSUM") as zps:
            for og2 in range(2):
                o_base = og2 * 1024
                pss = [None] * 8
                for dj in range(16):
                    raw = wop.tile([128, 1024], dt.float32, tag="raw")
                    nc.sync.dma_start(
                        out=raw[:], in_=woT_d[dj * 128:(dj + 1) * 128,
                                              o_base:o_base + 1024])
                    tnh = wop.tile([128, 1024], dt.bfloat16, tag="tnh")
                    nc.scalar.activation(tnh[:], raw[:], AF.Tanh)
                    for osub in range(2):
                        for tb in range(4):
                            k = osub * 4 + tb
                            if pss[k] is None:
                                pss[k] = zps.tile([128, 512], dt.float32,
                                                  tag=f"z{k}", name=f"z_{k}")
                            nc.tensor.matmul(
                                pss[k][:],
                                OT[:, dj, tb * 128:(tb + 1) * 128],
                                tnh[:, osub * 512:(osub + 1) * 512],
                                start=(dj == 0), stop=(dj == 15))
                for osub in range(2):
                    for tb in range(4):
                        sl = slice(o_base + osub * 512,
                                   o_base + osub * 512 + 512)
                        nc.vector.tensor_add(h2acc[tb][:, sl],
                                             pss[osub * 4 + tb][:],
                                             bo_bc[:, sl])
        with tc.tile_pool(name="xd", bufs=2) as xd, \
             tc.tile_pool(name="md", bufs=1) as md, \
             tc.tile_pool(name="trps2", bufs=4, space="PSUM") as trps2:
            for tb in range(4):
                for xh in range(2):
                    x_t = xd.tile([128, 1024], dt.float32, tag="x2")
                    nc.sync.dma_start(
                        out=x_t[:],
                        in_=xl_d[tb * 128:(tb + 1) * 128,
                                 xh * 1024:(xh + 1) * 1024])
                    nc.vector.tensor_add(
                        h2acc[tb][:, xh * 1024:(xh + 1) * 1024],
                        h2acc[tb][:, xh * 1024:(xh + 1) * 1024], x_t[:])
                nc.sync.dma_start(out=h2_d[tb * 128:(tb + 1) * 128, :],
                                  in_=h2acc[tb][:])
                m_t = md.tile([128, D], dt.float32, tag="m")
                layernorm(h2acc[tb], m_t, "g2", "b2")
                for dj in range(16):
                    ps = trps2.tile([128, 128], dt.float32, tag="tp2")
                    nc.tensor.transpose(ps[:], m_t[:, dj * 128:(dj + 1) * 128],
                                        ident[:])
                    nc.vector.tensor_copy(mT[:, dj, tb * 128:(tb + 1) * 128],
                                          ps[:])

        h2_pool.__exit__(None, None, None)

        # ---------- Phase E: MLP ----------
        gt_pool = tc.tile_pool(name="gtpl", bufs=1)
        gtpl = gt_pool.__enter__()
        GT1 = gtpl.tile([128, 32, 512], dt.bfloat16, name="GT1")
        GT2 = gtpl.tile([128, 32, 512], dt.bfloat16, name="GT2")

        def gt_slice(fti, c0, c1):
            if fti < 32:
                return GT1[:, fti, c0:c1]
            return GT2[:, fti - 32, c0:c1]

        if True:
            with tc.tile_pool(name="wfc", bufs=8) as wfcp, \
                 tc.tile_pool(name="ups", bufs=1, space="PSUM") as ups:
                for FG in range(8):            # 1024 f-cols per group
                    pss = [None] * 8
                    for dj in range(16):
                        wt = wfcp.tile([128, 1024], dt.bfloat16, tag="wfct")
                        nc.sync.dma_start(
                            out=wt[:],
                            in_=wfcT_v[dj * 128:(dj + 1) * 128,
                                       FG * 1024:(FG + 1) * 1024])
                        for fsub in range(8):
                            if pss[fsub] is None:
                                pss[fsub] = ups.tile([128, 512], dt.float32,
                                                     tag=f"u{fsub}",
                                                     name=f"u_{fsub}")
                            nc.tensor.matmul(
                                pss[fsub][:],
                                wt[:, fsub * 128:(fsub + 1) * 128],
                                mT[:, dj, :],
                                start=(dj == 0), stop=(dj == 15))
                    for fsub in range(8):
                        fti = FG * 8 + fsub
                        nc.scalar.activation(gt_slice(fti, 0, 512),
                                             pss[fsub][:],
                                             AF.Gelu_apprx_tanh,
                                             bias=bfc_pp[:, fti:fti + 1])
            with tc.tile_pool(name="wpj", bufs=5) as wpjp, \
                 tc.tile_pool(name="yps", bufs=1, space="PSUM") as yps, \
                 tc.tile_pool(name="outp", bufs=6) as outp:
                for tg in range(2):            # tt groups of 2
                    pss = {}
                    h2s_g = {}
                    for ft in range(64):
                        wt = wpjp.tile([128, D], dt.bfloat16, tag="wpjt")
                        nc.sync.dma_start(
                            out=wt[:], in_=wpjT_v[ft * 128:(ft + 1) * 128, :])
                        for ob in range(4):
                            for ti in range(2):
                                tt = tg * 2 + ti
                                key = (ob, ti)
                                if key not in pss:
                                    pss[key] = yps.tile(
                                        [128, 512], dt.float32,
                                        tag=f"y{ob}{ti}", name=f"y_{ob}_{ti}")
                                nc.tensor.matmul(
                                    pss[key][:],
                                    gt_slice(ft, tt * 128, (tt + 1) * 128),
                                    wt[:, ob * 512:(ob + 1) * 512],
                                    start=(ft == 0), stop=(ft == 63))
                    for ti in range(2):
                        tt = tg * 2 + ti
                        h2s = outp.tile([128, D], dt.float32, tag="h2s",
                                        bufs=2, name=f"h2s_{tt}")
                        nc.sync.dma_start(
                            out=h2s[:], in_=h2_d[tt * 128:(tt + 1) * 128, :])
                        h2s_g[ti] = h2s
                    for ob in range(4):
                        for ti in range(2):
                            tt = tg * 2 + ti
                            sl = slice(ob * 512, ob * 512 + 512)
                            o_t = outp.tile([128, 512], dt.float32, tag="o")
                            nc.vector.tensor_add(o_t[:], pss[(ob, ti)][:],
                                                 bpj_bc[:, sl])
                            nc.vector.tensor_add(o_t[:], o_t[:],
                                                 h2s_g[ti][:, sl])
                            nc.sync.dma_start(
                                out=out_d[tt * 128:(tt + 1) * 128, sl],
                                in_=o_t[:])
        gt_pool.__exit__(None, None, None)
        stack.close()

    nc.compile()
    return nc


def _host_prep(inputs):
    f32 = lambda k: np.ascontiguousarray(np.asarray(inputs[k], np.float32))
    x = f32("hidden_states")
    wqT = np.ascontiguousarray(f32("wq").T)
    wkT = np.ascontiguousarray(f32("wk").T)
    wvT = np.ascontiguousarray(f32("wv").T)
    woT = np.ascontiguousarray(f32("wo").T)
    wfcT = np.ascontiguousarray(f32("w_fc").T).ravel()
    wpjT = np.ascontiguousarray(f32("w_proj").T).ravel()
    kp = np.arange(128)
    q_f = np.arange(256)
    in_maps = []
    for c in range(NC):
        mask = np.empty((128, 4, 2, 512), np.float32)
        for jp in range(4):
            for tb in range(2):
                for half in range(2):
                    j = 2 * jp + half
                    ktok = 8 * (128 * tb + kp)[:, None] + j
                    qtok = 8 * q_f[None, :] + c
                    mask[:, jp, tb, half * 256:(half + 1) * 256] = np.where(
                        ktok <= qtok, 0.0, -1e9)
        in_maps.append({
            "xl": np.concatenate([x[0, c::NC, :], x[1, c::NC, :]], 0),
            "wqT": wqT, "wkT": wkT, "wvT": wvT, "woT": woT,
            "wfc_ch": wfcT[c * WFC_CH:(c + 1) * WFC_CH],
            "wpj_ch": wpjT[c * WPJ_CH:(c + 1) * WPJ_CH],
            "mask": mask.astype(ml_dtypes.bfloat16),
            "ln1g": f32("ln1_g"), "ln1b": f32("ln1_b"),
            "ln2g": f32("ln2_g"), "ln2b": f32("ln2_b"),
            "bo": f32("bo"), "bfc": f32("b_fc"), "bpj": f32("b_proj"),
        })
    return in_maps


def kernel(**inputs) -> np.ndarray:
    in_maps = _host_prep(inputs)
    key = (not bool(np.all(np.asarray(inputs["ln1_g"]) == 1.0)),
           not bool(np.all(np.asarray(inputs["ln1_b"]) == 0.0)),
           not bool(np.all(np.asarray(inputs["ln2_g"]) == 1.0)),
           not bool(np.all(np.asarray(inputs["ln2_b"]) == 0.0)))
    if key not in _CACHE:
        _CACHE[key] = _build(*key)
    nc = _CACHE[key]
    res = run_bass_kernel_spmd(nc, in_maps, core_ids=list(range(NC)))
    if res.exec_time_ns is not None:
        print(f"HW exec time: {res.exec_time_ns} ns")
    out = np.zeros((B, S, D), np.float32)
    for c in range(NC):
        o = res.results[c]["out"]
        out[0, c::NC] = o[:RPC]
        out[1, c::NC] = o[RPC:]
    return out



# revision 16
# speedup vs baseline: 1.1968x; 1.1968x over previous
"""BinaryGPTNeoBlock on 8 trn2 NeuronCores.

Sequence-parallel over 8 cores: core c owns rows {c, c+8, ...} of both
batches (256 per batch, 512 total); causality is per-core mask data so
the program stays SPMD-uniform. K/V are projected feature-/token-major
directly (no PE transposes), cast bf16, and AllGathered in two halves
each (interleaved with the projection passes) so attention starts with
no stall. MLP weights: each core tanh's + scales (x64) its 1/8th into
fp8, two AllGathers share them, and fc/proj run fp8 DoubleRow matmuls
(2x PE rate); the 1/64 descale folds into PSUM evacuation.

Self-contained: hardcodes shapes; host only shards/transposes/builds masks.
"""

import numpy as np
import ml_dtypes

import concourse.bass as bass
import concourse.tile as tile
from concourse import bacc, mybir
from concourse.bass_utils import run_bass_kernel_spmd
from concourse.masks import make_identity

B, S, D = 2, 2048, 2048
H = 16
HD = 128
FF = 4 * D
EPS = 1e-5
NC = 8
CH = 256               # q-chunk length (S // NC)
TL = 2 * CH            # 512 local rows (one chunk per batch)
WFC_CH = 256 * FF      # own d-rows of wfcT
WPJ_CH = 1024 * D      # own f-rows of wpjT
WS = 64.0              # fp8 weight pre-scale (undone at PSUM evacuation)

F8_MLP = True          # fc/proj in fp8 DoubleRow
F8_OP = False          # out-proj in fp8 DoubleRow

dt = mybir.dt
AF = mybir.ActivationFunctionType
OP = mybir.AluOpType
DR = mybir.MatmulPerfMode.DoubleRow

_CACHE = {}


def _build(apply_g1, apply_b1, apply_g2, apply_b2):
    nc = bacc.Bacc("TRN2", target_bir_lowering=False, debug=False,
                   num_devices=NC)

    xl_d = nc.dram_tensor("xl", [TL, D], dt.float32, kind="ExternalInput").ap()
    wqT_d = nc.dram_tensor("wqT", [D, D], dt.float32, kind="ExternalInput").ap()
    wkT_d = nc.dram_tensor("wkT", [D, D], dt.float32, kind="ExternalInput").ap()
    wvT_d = nc.dram_tensor("wvT", [D, D], dt.float32, kind="ExternalInput").ap()
    woT_d = nc.dram_tensor("woT", [D, D], dt.float32, kind="ExternalInput").ap()
    wfc_ch_d = nc.dram_tensor("wfc_ch", [WFC_CH], dt.float32,
                              kind="ExternalInput").ap()
    wpj_ch_d = nc.dram_tensor("wpj_ch", [WPJ_CH], dt.float32,
                              kind="ExternalInput").ap()
    mask_d = nc.dram_tensor("mask", [128, 8, 2, 256], dt.bfloat16,
                            kind="ExternalInput").ap()
    ln1g_d = nc.dram_tensor("ln1g", [D], dt.float32, kind="ExternalInput").ap()
    ln1b_d = nc.dram_tensor("ln1b", [D], dt.float32, kind="ExternalInput").ap()
    ln2g_d = nc.dram_tensor("ln2g", [D], dt.float32, kind="ExternalInput").ap()
    ln2b_d = nc.dram_tensor("ln2b", [D], dt.float32, kind="ExternalInput").ap()
    bo_d = nc.dram_tensor("bo", [D], dt.float32, kind="ExternalInput").ap()
    bfc_d = nc.dram_tensor("bfc", [FF], dt.float32, kind="ExternalInput").ap()
    bpj_d = nc.dram_tensor("bpj", [D], dt.float32, kind="ExternalInput").ap()
    out_d = nc.dram_tensor("out", [TL, D], dt.float32,
                           kind="ExternalOutput").ap()

    mlp_dt = dt.float8e4 if F8_MLP else dt.bfloat16
    op_dt = dt.float8e4 if F8_OP else dt.bfloat16
    KHALF = 1024 * TL        # elems of one kT half per core
    VHALF = TL * 1024

    def bcast_row(src_ap, n):
        return bass.AP(tensor=src_ap.tensor, offset=src_ap.offset,
                       ap=[[0, 128], [1, n]])

    with tile.TileContext(nc) as tc:
        import contextlib
        stack = contextlib.ExitStack()
        main = stack.enter_context(tc.tile_pool(name="main", bufs=1))
        dram = stack.enter_context(
            tc.tile_pool(name="dram", bufs=1, space="DRAM"))

        ident = main.tile([128, 128], dt.float32)
        make_identity(nc, ident[:])
        ones_col_b = main.tile([128, 1], dt.bfloat16)
        nc.vector.memset(ones_col_b[:], 1.0)
        ones_row = main.tile([1, 128], dt.float32)
        nc.vector.memset(ones_row[:], 1.0)
        eps_t = main.tile([128, 1], dt.float32)
        nc.vector.memset(eps_t[:], EPS)
        bo_bc = main.tile([128, D], dt.float32)
        nc.sync.dma_start(out=bo_bc[:], in_=bcast_row(bo_d, D))
        bpj_bc = main.tile([128, D], dt.float32)
        nc.sync.dma_start(out=bpj_bc[:], in_=bcast_row(bpj_d, D))
        masks = main.tile([128, 8, 2, 256], dt.bfloat16)
        nc.sync.dma_start(out=masks[:], in_=mask_d[:])
        bfc_pp = main.tile([128, FF // 128], dt.float32)
        nc.sync.dma_start(
            out=bfc_pp[:],
            in_=bass.AP(tensor=bfc_d.tensor, offset=bfc_d.offset,
                        ap=[[1, 128], [128, FF // 128]]))
        ln_bc = {}
        for nm, flag, src in (("g1", apply_g1, ln1g_d),
                              ("b1", apply_b1, ln1b_d),
                              ("g2", apply_g2, ln2g_d),
                              ("b2", apply_b2, ln2b_d)):
            if flag:
                t = main.tile([128, D], dt.float32, name=f"ln_{nm}")
                nc.sync.dma_start(out=t[:], in_=bcast_row(src, D))
                ln_bc[nm] = t

        # rotating big activation slots (16KB/part each, 2 slots)
        hT = main.tile([128, 16, 512], dt.bfloat16, tag="bigA", bufs=2,
                       name="hT")
        QT = main.tile([128, 16, 512], dt.bfloat16, tag="bigA", bufs=2,
                       name="QT")

        # ------- MLP weight prep (own 1/8) -> fp8 bounce, first so its
        # SBUF frees early; AG calls are issued later (after K/V AGs) ---
        wfc_bounce = dram.tile([WFC_CH], mlp_dt)
        wpj_bounce = dram.tile([WPJ_CH], mlp_dt)
        wfc_gath = dram.tile([NC * WFC_CH], mlp_dt, addr_space="Shared")
        wpj_gath = dram.tile([NC * WPJ_CH], mlp_dt, addr_space="Shared")
        with tc.tile_pool(name="wprep", bufs=3) as wprep:
            for src, dst, nrb, ncols in ((wfc_ch_d, wfc_bounce, 2, FF),
                                         (wpj_ch_d, wpj_bounce, 8, D)):
                for rb in range(nrb):
                    for ci in range(ncols // 2048):
                        off = rb * 128 * ncols + ci * 2048
                        raw = wprep.tile([128, 2048], dt.float32, tag="wraw")
                        nc.sync.dma_start(
                            out=raw[:],
                            in_=bass.AP(tensor=src.tensor,
                                        offset=src.offset + off,
                                        ap=[[ncols, 128], [1, 2048]]))
                        tnh = wprep.tile([128, 2048], dt.float32, tag="wtnh")
                        nc.scalar.activation(tnh[:], raw[:], AF.Tanh)
                        w8 = wprep.tile([128, 2048], mlp_dt, tag="w8")
                        if F8_MLP:
                            nc.vector.tensor_scalar(w8[:], tnh[:], WS, None,
                                                    op0=OP.mult)
                        else:
                            nc.vector.tensor_copy(w8[:], tnh[:])
                        nc.sync.dma_start(
                            out=bass.AP(tensor=dst.tensor,
                                        offset=dst.offset + off,
                                        ap=[[ncols, 128], [1, 2048]]),
                            in_=w8[:])

        # ---------- Phase A: x -> LN1 -> h^T ----------
        def layernorm(x_t, h_t, gk, bk):
            with tc.tile_pool(name="lnp", bufs=2) as lp:
                st = lp.tile([128, 4, 6], dt.float32, tag="st")
                xr = x_t[:].rearrange("p (n f) -> p n f", n=4)
                for sg in range(4):
                    nc.vector.bn_stats(out=st[:, sg, :], in_=xr[:, sg, :])
                mv = lp.tile([128, 2], dt.float32, tag="mv")
                nc.vector.bn_aggr(out=mv[:], in_=st[:])
                std = lp.tile([128, 1], dt.float32, tag="sd")
                nc.scalar.activation(std[:], mv[:, 1:2], AF.Sqrt,
                                     bias=eps_t[:])
                rstd = lp.tile([128, 1], dt.float32, tag="rs")
                nc.vector.reciprocal(rstd[:], std[:])
                nc.vector.tensor_scalar(h_t[:], x_t[:], mv[:, 0:1], rstd[:],
                                        op0=OP.subtract, op1=OP.mult)
                if gk in ln_bc:
                    nc.vector.tensor_mul(h_t[:], h_t[:], ln_bc[gk][:])
                if bk in ln_bc:
                    nc.vector.tensor_add(h_t[:], h_t[:], ln_bc[bk][:])

        with tc.tile_pool(name="xa", bufs=2) as xa, \
             tc.tile_pool(name="ha", bufs=2) as ha, \
             tc.tile_pool(name="trps", bufs=4, space="PSUM") as trps:
            for tb in range(4):
                x_t = xa.tile([128, D], dt.float32, tag="x")
                nc.sync.dma_start(out=x_t[:],
                                  in_=xl_d[tb * 128:(tb + 1) * 128, :])
                h_t = ha.tile([128, D], dt.float32, tag="h")
                layernorm(x_t, h_t, "g1", "b1")
                for dj in range(16):
                    ps = trps.tile([128, 128], dt.float32, tag="tp")
                    nc.tensor.transpose(ps[:], h_t[:, dj * 128:(dj + 1) * 128],
                                        ident[:])
                    nc.vector.tensor_copy(hT[:, dj, tb * 128:(tb + 1) * 128],
                                          ps[:])

        # ---------- Phase B: QKV (feature-major K/Q, token-major V) ----
        k_bounce = [dram.tile([KHALF], dt.bfloat16, name=f"kb{i}")
                    for i in range(2)]
        v_bounce = [dram.tile([VHALF], dt.bfloat16, name=f"vb{i}")
                    for i in range(2)]
        k_gath = [dram.tile([NC * KHALF], dt.bfloat16, addr_space="Shared",
                            name=f"kg{i}") for i in range(2)]
        v_gath = [dram.tile([NC * VHALF], dt.bfloat16, addr_space="Shared",
                            name=f"vg{i}") for i in range(2)]

        qkv_pool = tc.tile_pool(name="qkv", bufs=3)
        qkvp = qkv_pool.__enter__()
        qkv_ps_pool = tc.tile_pool(name="qkvps", bufs=1, space="PSUM")
        qkvps = qkv_ps_pool.__enter__()
        kacc_pool = tc.tile_pool(name="kacc", bufs=1)
        kaccp = kacc_pool.__enter__()
        kacc = kaccp.tile([128, 16, 512], dt.bfloat16, name="kacc")
        vacc = [kaccp.tile([128, D], dt.bfloat16, name=f"vacc{t}")
                for t in range(4)]

        def proj_fmajor(wT_dram, pss, dest, bank):
            # features [pss*512, pss*512+512) of w^T h^T -> dest[:, 4pss..]
            ps = [qkvps.tile([128, 512], dt.float32, tag=f"q{bank * 4 + i}",
                             name=f"ps_{wT_dram.tensor.name}_{pss}_{i}")
                  for i in range(4)]
            for dj in range(16):
                raw = qkvp.tile([128, 512], dt.float32, tag="qkraw")
                nc.sync.dma_start(
                    out=raw[:],
                    in_=wT_dram[dj * 128:(dj + 1) * 128,
                                pss * 512:(pss + 1) * 512])
                wt = qkvp.tile([128, 512], dt.bfloat16, tag="qktnh")
                nc.scalar.activation(wt[:], raw[:], AF.Tanh)
                for ft in range(4):
                    nc.tensor.matmul(ps[ft][:],
                                     wt[:, ft * 128:(ft + 1) * 128],
                                     hT[:, dj, :],
                                     start=(dj == 0), stop=(dj == 15))
            for ft in range(4):
                nc.vector.tensor_copy(dest[:, pss * 4 + ft, :], ps[ft][:])

        def proj_v(fgp, bank):
            # token-major v for features [fgp*1024, fgp*1024+1024)
            ps = [qkvps.tile([128, 512], dt.float32, tag=f"q{i}",
                             name=f"ps_v_{fgp}_{i}")
                  for i in range(8)]
            for dj in range(16):
                raw = qkvp.tile([128, 1024], dt.float32, tag="vraw")
                nc.sync.dma_start(
                    out=raw[:],
                    in_=wvT_d[dj * 128:(dj + 1) * 128,
                              fgp * 1024:(fgp + 1) * 1024])
                wt = qkvp.tile([128, 1024], dt.bfloat16, tag="vtnh")
                nc.scalar.activation(wt[:], raw[:], AF.Tanh)
                for tt in range(4):
                    for fg2 in range(2):
                        nc.tensor.matmul(
                            ps[tt * 2 + fg2][:],
                            hT[:, dj, tt * 128:(tt + 1) * 128],
                            wt[:, fg2 * 512:(fg2 + 1) * 512],
                            start=(dj == 0), stop=(dj == 15))
            for tt in range(4):
                for fg2 in range(2):
                    nc.vector.tensor_copy(
                        vacc[tt][:, fgp * 1024 + fg2 * 512:
                                 fgp * 1024 + fg2 * 512 + 512],
                        ps[tt * 2 + fg2][:])

        def dump_k(half):
            nc.sync.dma_start(
                out=bass.AP(tensor=k_bounce[half].tensor,
                            offset=k_bounce[half].offset,
                            ap=[[512, 128], [128 * 512, 8], [1, 512]]),
                in_=kacc[:, half * 8:(half + 1) * 8, :])

        def dump_v(fgp):
            for tt in range(4):
                nc.sync.dma_start(
                    out=bass.AP(tensor=v_bounce[fgp].tensor,
                                offset=v_bounce[fgp].offset + tt * 128 * 1024,
                                ap=[[1024, 128], [1, 1024]]),
                    in_=vacc[tt][:, fgp * 1024:(fgp + 1) * 1024])

        def ag(src, dst):
            nc.gpsimd.collective_compute(
                "AllGather", OP.bypass, replica_groups=[list(range(NC))],
                ins=[src[:]], outs=[dst[:]])

        proj_fmajor(wkT_d, 0, kacc, 0)
        proj_fmajor(wkT_d, 1, kacc, 1)
        dump_k(0)
        ag(k_bounce[0], k_gath[0])
        proj_v(0, 0)
        dump_v(0)
        ag(v_bounce[0], v_gath[0])
        proj_fmajor(wkT_d, 2, kacc, 1)
        proj_fmajor(wkT_d, 3, kacc, 0)
        dump_k(1)
        ag(k_bounce[1], k_gath[1])
        proj_v(1, 1)
        dump_v(1)
        ag(v_bounce[1], v_gath[1])
        ag(wfc_bounce, wfc_gath)
        ag(wpj_bounce, wpj_gath)
        for pss in range(4):
            proj_fmajor(wqT_d, pss, QT, pss % 2)

        kacc_pool.__exit__(None, None, None)
        qkv_ps_pool.__exit__(None, None, None)
        qkv_pool.__exit__(None, None, None)

        # ---------- Phase C: attention (causality via per-core masks) ----
        # rank r's gather block holds its 512 local tokens (256/batch,
        # rows {r, r+8, ...}); every core scans all 8 ranks per batch.
        OT = main.tile([128, 16, 512], op_dt, tag="bigA", bufs=2, name="OT")
        with tc.tile_pool(name="kvh", bufs=1) as kvh, \
             tc.tile_pool(name="att", bufs=6) as att, \
             tc.tile_pool(name="stps", bufs=3, space="PSUM") as stps, \
             tc.tile_pool(name="otps", bufs=2, space="PSUM") as otps, \
             tc.tile_pool(name="dnps", bufs=2, space="PSUM") as dnps, \
             tc.tile_pool(name="bcps", bufs=1, space="PSUM") as bcps:
            for hg in range(4):
                gh = hg // 2          # K/V gather half holding this hg
                ho = (hg % 2) * 4 * 128   # head offset inside the half
                kt_g, v_g = [], []
                for r in range(NC):
                    kt = kvh.tile([128, 4, 512], dt.bfloat16, tag="kth",
                                  bufs=11, name=f"kt_{hg}_{r}")
                    nc.sync.dma_start(
                        out=kt[:],
                        in_=bass.AP(
                            tensor=k_gath[gh].tensor,
                            offset=k_gath[gh].offset + r * KHALF
                            + ho * 512,
                            ap=[[512, 128], [128 * 512, 4], [1, 512]]))
                    kt_g.append(kt)
                    vt = kvh.tile([128, 4, 4, 128], dt.bfloat16,
                                  tag="vth", bufs=11, name=f"vt_{hg}_{r}")
                    nc.sync.dma_start(
                        out=vt[:],
                        in_=bass.AP(
                            tensor=v_gath[gh].tensor,
                            offset=v_gath[gh].offset + r * VHALF + ho,
                            ap=[[1024, 128], [128 * 1024, 4],
                                [128, 4], [1, 128]]))
                    v_g.append(vt)
                for b in range(2):
                    qoff = b * 256
                    for hh in range(4):
                        h = hg * 4 + hh
                        ot_ps = otps.tile([128, 256], dt.float32, tag="ot")
                        dn_ps = dnps.tile([1, 256], dt.float32, tag="dn")
                        for r in range(NC):
                            st = stps.tile([128, 2, 256], dt.float32,
                                           tag="st")
                            for ksub in range(2):
                                nc.tensor.matmul(
                                    st[:, ksub, :],
                                    kt_g[r][:, hh,
                                            qoff + ksub * 128:
                                            qoff + ksub * 128 + 128],
                                    QT[:, h, qoff:qoff + 256],
                                    start=True, stop=True)
                            ptp = att.tile([128, 2, 256], dt.bfloat16,
                                           tag="ptp")
                            nc.vector.tensor_add(ptp[:], st[:],
                                                 masks[:, r, :, :])
                            pt = att.tile([128, 2, 256], dt.bfloat16,
                                          tag="pt")
                            nc.scalar.activation(pt[:], ptp[:], AF.Exp)
                            for ksub in range(2):
                                last = (r == NC - 1 and ksub == 1)
                                first = (r == 0 and ksub == 0)
                                nc.tensor.matmul(
                                    ot_ps[:],
                                    v_g[r][:, b * 2 + ksub, hh, :],
                                    pt[:, ksub, :],
                                    start=first, stop=last,
                                    skip_group_check=True)
                                nc.tensor.matmul(
                                    dn_ps[:], ones_col_b[:],
                                    pt[:, ksub, :],
                                    start=first, stop=last,
                                    skip_group_check=True)
                        dn_sb = att.tile([1, 256], dt.float32, tag="dns")
                        nc.vector.reciprocal(dn_sb[:], dn_ps[:])
                        bc_ps = bcps.tile([128, 256], dt.float32, tag="bc")
                        nc.tensor.matmul(bc_ps[:], ones_row[:], dn_sb[:],
                                         start=True, stop=True)
                        bc_sb = att.tile([128, 256], dt.float32, tag="bcs")
                        nc.vector.tensor_copy(bc_sb[:], bc_ps[:])
                        nc.vector.tensor_mul(OT[:, h, qoff:qoff + 256],
                                             ot_ps[:], bc_sb[:])

        # ---------- Phase D: out-proj + residual -> h2; LN2 -> m^T ----
        h2_pool = tc.tile_pool(name="h2a", bufs=1)
        h2a = h2_pool.__enter__()
        h2acc = [h2a.tile([128, D], dt.float32, name=f"h2_{t}")
                 for t in range(4)]
        KS = 2 if F8_OP else 1
        with tc.tile_pool(name="wo", bufs=3) as wop, \
             tc.tile_pool(name="xd", bufs=3) as xd, \
             tc.tile_pool(name="dps", bufs=1, space="PSUM") as dps:
            for dgp in range(2):
                ps = [dps.tile([128, 512], dt.float32, tag=f"d{i}",
                               name=f"dp_{dgp}_{i}") for i in range(8)]
                for og in range(16 // KS):
                    raw = wop.tile([128, KS, 1024], dt.float32, tag="oraw")
                    nc.sync.dma_start(
                        out=raw[:],
                        in_=bass.AP(tensor=woT_d.tensor,
                                    offset=woT_d.offset
                                    + og * KS * 128 * D + dgp * 1024,
                                    ap=[[D, 128], [128 * D, KS], [1, 1024]]))
                    if F8_OP:
                        tnh = wop.tile([128, KS, 1024], dt.float32,
                                       tag="otnh32")
                        nc.scalar.activation(tnh[:], raw[:], AF.Tanh)
                        wt = wop.tile([128, KS, 1024], op_dt, tag="otnh")
                        nc.vector.tensor_scalar(wt[:], tnh[:], WS, None,
                                                op0=OP.mult)
                    else:
                        wt = wop.tile([128, KS, 1024], dt.bfloat16,
                                      tag="otnh")
                        nc.scalar.activation(wt[:], raw[:], AF.Tanh)
                    for tt in range(4):
                        for dg2 in range(2):
                            if F8_OP:
                                nc.tensor.matmul(
                                    ps[tt * 2 + dg2][:],
                                    OT[:, og * 2:og * 2 + 2,
                                       tt * 128:(tt + 1) * 128],
                                    wt[:, :, dg2 * 512:(dg2 + 1) * 512],
                                    start=(og == 0), stop=(og == 7),
                                    perf_mode=DR)
                            else:
                                nc.tensor.matmul(
                                    ps[tt * 2 + dg2][:],
                                    OT[:, og, tt * 128:(tt + 1) * 128],
                                    wt[:, 0, dg2 * 512:(dg2 + 1) * 512],
                                    start=(og == 0), stop=(og == 15))
                for tt in range(4):
                    x_t = xd.tile([128, 1024], dt.float32, tag="x2")
                    nc.sync.dma_start(
                        out=x_t[:],
                        in_=xl_d[tt * 128:(tt + 1) * 128,
                                 dgp * 1024:(dgp + 1) * 1024])
                    for dg2 in range(2):
                        sl = slice(dgp * 1024 + dg2 * 512,
                                   dgp * 1024 + dg2 * 512 + 512)
                        if F8_OP:
                            nc.vector.tensor_scalar(
                                h2acc[tt][:, sl], ps[tt * 2 + dg2][:],
                                1.0 / WS, None, op0=OP.mult)
                            nc.vector.tensor_add(h2acc[tt][:, sl],
                                                 h2acc[tt][:, sl],
                                                 bo_bc[:, sl])
                        else:
                            nc.vector.tensor_add(h2acc[tt][:, sl],
                                                 ps[tt * 2 + dg2][:],
                                                 bo_bc[:, sl])
                        nc.vector.tensor_add(
                            h2acc[tt][:, sl], h2acc[tt][:, sl],
                            x_t[:, dg2 * 512:(dg2 + 1) * 512])

        mT = main.tile([128, 16, 512], mlp_dt, tag="bigA", bufs=2, name="mT")
        with tc.tile_pool(name="md", bufs=2) as md, \
             tc.tile_pool(name="trps2", bufs=4, space="PSUM") as trps2:
            for tb in range(4):
                m_t = md.tile([128, D], dt.float32, tag="m")
                layernorm(h2acc[tb], m_t, "g2", "b2")
                for dj in range(16):
                    ps = trps2.tile([128, 128], dt.float32, tag="tp2")
                    nc.tensor.transpose(ps[:], m_t[:, dj * 128:(dj + 1) * 128],
                                        ident[:])
                    nc.vector.tensor_copy(mT[:, dj, tb * 128:(tb + 1) * 128],
                                          ps[:])

        # ---------- Phase E: MLP (fp8 DoubleRow) ----------
        gt_pool = tc.tile_pool(name="gtpl", bufs=1)
        gtpl = gt_pool.__enter__()
        GT = gtpl.tile([128, 64, 512], mlp_dt, name="GT")
        wfcT_v = wfc_gath
        wpjT_v = wpj_gath
        MKS = 2 if F8_MLP else 1

        with tc.tile_pool(name="wfc", bufs=6) as wfcp, \
             tc.tile_pool(name="ups", bufs=2, space="PSUM") as ups:
            for grp in range(16):        # 4 f-tiles (512 features) per group
                ps = [ups.tile([128, 512], dt.float32, tag=f"u{i}",
                               name=f"u_{grp}_{i}") for i in range(4)]
                for djp in range(16 // MKS):
                    w2 = wfcp.tile([128, MKS, 512], mlp_dt, tag="wfct")
                    nc.sync.dma_start(
                        out=w2[:],
                        in_=bass.AP(tensor=wfcT_v.tensor,
                                    offset=wfcT_v.offset
                                    + djp * MKS * 128 * FF + grp * 512,
                                    ap=[[FF, 128], [128 * FF, MKS],
                                        [1, 512]]))
                    for f4 in range(4):
                        if F8_MLP:
                            nc.tensor.matmul(
                                ps[f4][:],
                                w2[:, :, f4 * 128:(f4 + 1) * 128],
                                mT[:, djp * 2:djp * 2 + 2, :],
                                start=(djp == 0), stop=(djp == 7),
                                perf_mode=DR)
                        else:
                            nc.tensor.matmul(
                                ps[f4][:],
                                w2[:, 0, f4 * 128:(f4 + 1) * 128],
                                mT[:, djp, :],
                                start=(djp == 0), stop=(djp == 15))
                for f4 in range(4):
                    fti = grp * 4 + f4
                    nc.scalar.activation(GT[:, fti, :], ps[f4][:],
                                         AF.Gelu_apprx_tanh,
                                         bias=bfc_pp[:, fti:fti + 1],
                                         scale=(1.0 / WS if F8_MLP else 1.0))

        with tc.tile_pool(name="wpj", bufs=4) as wpjp, \
             tc.tile_pool(name="yps", bufs=1, space="PSUM") as yps, \
             tc.tile_pool(name="outp", bufs=4) as outp:
            for ttp in range(2):
                ps = [yps.tile([128, 512], dt.float32, tag=f"y{i}",
                               name=f"y_{ttp}_{i}") for i in range(8)]
                for fp in range(64 // MKS):
                    wp2 = wpjp.tile([128, MKS, 2048], mlp_dt, tag="wpjt")
                    nc.sync.dma_start(
                        out=wp2[:],
                        in_=bass.AP(tensor=wpjT_v.tensor,
                                    offset=wpjT_v.offset
                                    + fp * MKS * 128 * D,
                                    ap=[[D, 128], [128 * D, MKS],
                                        [1, 2048]]))
                    for tt2 in range(2):
                        tt = ttp * 2 + tt2
                        for dg in range(4):
                            if F8_MLP:
                                nc.tensor.matmul(
                                    ps[tt2 * 4 + dg][:],
                                    GT[:, fp * 2:fp * 2 + 2,
                                       tt * 128:(tt + 1) * 128],
                                    wp2[:, :, dg * 512:(dg + 1) * 512],
                                    start=(fp == 0), stop=(fp == 31),
                                    perf_mode=DR)
                            else:
                                nc.tensor.matmul(
                                    ps[tt2 * 4 + dg][:],
                                    GT[:, fp, tt * 128:(tt + 1) * 128],
                                    wp2[:, 0, dg * 512:(dg + 1) * 512],
                                    start=(fp == 0), stop=(fp == 63))
                for tt2 in range(2):
                    tt = ttp * 2 + tt2
                    for dg in range(4):
                        sl = slice(dg * 512, dg * 512 + 512)
                        o_t = outp.tile([128, 512], dt.float32, tag="o")
                        if F8_MLP:
                            nc.vector.tensor_scalar(
                                o_t[:], ps[tt2 * 4 + dg][:], 1.0 / WS, None,
                                op0=OP.mult)
                            nc.vector.tensor_add(o_t[:], o_t[:],
                                                 bpj_bc[:, sl])
                        else:
                            nc.vector.tensor_add(o_t[:], ps[tt2 * 4 + dg][:],
                                                 bpj_bc[:, sl])
                        nc.vector.tensor_add(o_t[:], o_t[:],
                                             h2acc[tt][:, sl])
                        nc.sync.dma_start(
                            out=out_d[tt * 128:(tt + 1) * 128, sl],
                            in_=o_t[:])
        gt_pool.__exit__(None, None, None)
        h2_pool.__exit__(None, None, None)
        stack.close()

    nc.compile()
    return nc


def _host_prep(inputs):
    f32 = lambda k: np.ascontiguousarray(np.asarray(inputs[k], np.float32))
    x = f32("hidden_states")
    wqT = np.ascontiguousarray(f32("wq").T)
    wkT = np.ascontiguousarray(f32("wk").T)
    wvT = np.ascontiguousarray(f32("wv").T)
    woT = np.ascontiguousarray(f32("wo").T)
    wfcT = np.ascontiguousarray(f32("w_fc").T).ravel()
    wpjT = np.ascontiguousarray(f32("w_proj").T).ravel()
    # causal masks per core: q token = 8*qf + c, k token = 8*(ks*128+kp) + r
    kp = np.arange(128)[:, None, None, None]
    rr = np.arange(8)[None, :, None, None]
    ks = np.arange(2)[None, None, :, None]
    qf = np.arange(256)[None, None, None, :]
    in_maps = []
    for c in range(NC):
        mask = np.where(8 * (ks * 128 + kp) + rr <= 8 * qf + c,
                        0.0, -1e9).astype(np.float32)
        mask = mask.astype(ml_dtypes.bfloat16)
        in_maps.append({
            "xl": np.concatenate([x[0, c::NC, :], x[1, c::NC, :]], 0),
            "wqT": wqT, "wkT": wkT, "wvT": wvT, "woT": woT,
            "wfc_ch": wfcT[c * WFC_CH:(c + 1) * WFC_CH],
            "wpj_ch": wpjT[c * WPJ_CH:(c + 1) * WPJ_CH],
            "mask": mask,
            "ln1g": f32("ln1_g"), "ln1b": f32("ln1_b"),
            "ln2g": f32("ln2_g"), "ln2b": f32("ln2_b"),
            "bo": f32("bo"), "bfc": f32("b_fc"), "bpj": f32("b_proj"),
        })
    return in_maps


def kernel(**inputs) -> np.ndarray:
    in_maps = _host_prep(inputs)
    key = (not bool(np.all(np.asarray(inputs["ln1_g"]) == 1.0)),
           not bool(np.all(np.asarray(inputs["ln1_b"]) == 0.0)),
           not bool(np.all(np.asarray(inputs["ln2_g"]) == 1.0)),
           not bool(np.all(np.asarray(inputs["ln2_b"]) == 0.0)))
    if key not in _CACHE:
        _CACHE[key] = _build(*key)
    nc = _CACHE[key]
    res = run_bass_kernel_spmd(nc, in_maps, core_ids=list(range(NC)))
    if res.exec_time_ns is not None:
        print(f"HW exec time: {res.exec_time_ns} ns")
    out = np.zeros((B, S, D), np.float32)
    for c in range(NC):
        o = res.results[c]["out"]
        out[0, c::NC] = o[:CH]
        out[1, c::NC] = o[CH:]
    return out


# revision 24
# speedup vs baseline: 1.3041x; 1.0896x over previous
"""BinaryGPTNeoBlock on 8 trn2 NeuronCores.

Sequence-parallel over 8 cores: core c owns rows {c, c+8, ...} of both
batches (256 per batch, 512 total); causality is per-core mask data so
the program stays SPMD-uniform. K/V are projected feature-/token-major
directly (no PE transposes), cast bf16, and AllGathered in two halves
each (interleaved with the projection passes) so attention starts with
no stall. MLP weights: each core tanh's + scales (x64) its 1/8th into
fp8, two AllGathers share them, and fc/proj run fp8 DoubleRow matmuls
(2x PE rate); the 1/64 descale folds into PSUM evacuation.

Self-contained: hardcodes shapes; host only shards/transposes/builds masks.
"""

import numpy as np
import ml_dtypes

import concourse.bass as bass
import concourse.tile as tile
from concourse import bacc, mybir
from concourse.bass_utils import run_bass_kernel_spmd
from concourse.masks import make_identity

B, S, D = 2, 2048, 2048
H = 16
HD = 128
FF = 4 * D
EPS = 1e-5
NC = 8
CH = 256               # q-chunk length (S // NC)
TL = 2 * CH            # 512 local rows (one chunk per batch)
WFC_CH = 256 * FF      # own d-rows of wfcT
WPJ_CH = 1024 * D      # own f-rows of wpjT
WS = 64.0              # fp8 weight pre-scale (undone at PSUM evacuation)

F8_MLP = True          # fc/proj in fp8 DoubleRow
F8_OP = False          # out-proj in fp8 DoubleRow

dt = mybir.dt
AF = mybir.ActivationFunctionType
OP = mybir.AluOpType
DR = mybir.MatmulPerfMode.DoubleRow

_CACHE = {}


def _build(apply_g1, apply_b1, apply_g2, apply_b2):
    nc = bacc.Bacc("TRN2", target_bir_lowering=False, debug=False,
                   num_devices=NC)

    xl_d = nc.dram_tensor("xl", [TL, D], dt.float32, kind="ExternalInput").ap()
    wqT_d = nc.dram_tensor("wqT", [D, D], dt.float32, kind="ExternalInput").ap()
    wkT_d = nc.dram_tensor("wkT", [D, D], dt.float32, kind="ExternalInput").ap()
    wvT_d = nc.dram_tensor("wvT", [D, D], dt.float32, kind="ExternalInput").ap()
    woT_d = nc.dram_tensor("woT", [D, D], dt.float32, kind="ExternalInput").ap()
    wfc_ch_d = nc.dram_tensor("wfc_ch", [WFC_CH], dt.float32,
                              kind="ExternalInput").ap()
    wpj_ch_d = nc.dram_tensor("wpj_ch", [WPJ_CH], dt.float32,
                              kind="ExternalInput").ap()
    mask_d = nc.dram_tensor("mask", [128, 8, 384], dt.bfloat16,
                            kind="ExternalInput").ap()
    ln1g_d = nc.dram_tensor("ln1g", [D], dt.float32, kind="ExternalInput").ap()
    ln1b_d = nc.dram_tensor("ln1b", [D], dt.float32, kind="ExternalInput").ap()
    ln2g_d = nc.dram_tensor("ln2g", [D], dt.float32, kind="ExternalInput").ap()
    ln2b_d = nc.dram_tensor("ln2b", [D], dt.float32, kind="ExternalInput").ap()
    bo_d = nc.dram_tensor("bo", [D], dt.float32, kind="ExternalInput").ap()
    bfc_d = nc.dram_tensor("bfc", [FF], dt.float32, kind="ExternalInput").ap()
    bpj_d = nc.dram_tensor("bpj", [D], dt.float32, kind="ExternalInput").ap()
    out_d = nc.dram_tensor("out", [TL, D], dt.float32,
                           kind="ExternalOutput").ap()

    mlp_dt = dt.float8e4 if F8_MLP else dt.bfloat16
    op_dt = dt.float8e4 if F8_OP else dt.bfloat16
    KHALF = 1024 * TL        # elems of one kT half per core
    VHALF = TL * 1024

    def bcast_row(src_ap, n):
        return bass.AP(tensor=src_ap.tensor, offset=src_ap.offset,
                       ap=[[0, 128], [1, n]])

    with tile.TileContext(nc) as tc:
        import contextlib
        stack = contextlib.ExitStack()
        main = stack.enter_context(tc.tile_pool(name="main", bufs=1))
        dram = stack.enter_context(
            tc.tile_pool(name="dram", bufs=1, space="DRAM"))

        ident = main.tile([128, 128], dt.float32)
        make_identity(nc, ident[:])
        ident_b = main.tile([128, 128], dt.bfloat16)
        nc.vector.tensor_copy(ident_b[:], ident[:])
        ones_col_b = main.tile([128, 1], dt.bfloat16)
        nc.vector.memset(ones_col_b[:], 1.0)
        ones_row = main.tile([1, 128], dt.float32)
        nc.vector.memset(ones_row[:], 1.0)
        eps_t = main.tile([128, 1], dt.float32)
        nc.vector.memset(eps_t[:], EPS)
        bo_bc = main.tile([128, D], dt.float32)
        nc.sync.dma_start(out=bo_bc[:], in_=bcast_row(bo_d, D))
        bpj_bc = main.tile([128, D], dt.float32)
        nc.sync.dma_start(out=bpj_bc[:], in_=bcast_row(bpj_d, D))
        masks = main.tile([128, 8, 384], dt.bfloat16)
        nc.sync.dma_start(out=masks[:], in_=mask_d[:])
        bfc_pp = main.tile([128, FF // 128], dt.float32)
        nc.sync.dma_start(
            out=bfc_pp[:],
            in_=bass.AP(tensor=bfc_d.tensor, offset=bfc_d.offset,
                        ap=[[1, 128], [128, FF // 128]]))
        ln_bc = {}
        for nm, flag, src in (("g1", apply_g1, ln1g_d),
                              ("b1", apply_b1, ln1b_d),
                              ("g2", apply_g2, ln2g_d),
                              ("b2", apply_b2, ln2b_d)):
            if flag:
                t = main.tile([128, D], dt.float32, name=f"ln_{nm}")
                nc.sync.dma_start(out=t[:], in_=bcast_row(src, D))
                ln_bc[nm] = t

        # rotating big activation slots (16KB/part each, 2 slots)
        hT = main.tile([128, 16, 512], dt.bfloat16, tag="bigA", bufs=2,
                       name="hT")
        QT = main.tile([128, 16, 512], dt.bfloat16, tag="bigA", bufs=2,
                       name="QT")

        wfc_bounce = dram.tile([WFC_CH], mlp_dt)
        wpj_bounce = dram.tile([WPJ_CH], mlp_dt)
        wfc_gath = dram.tile([NC * WFC_CH], mlp_dt, addr_space="Shared")
        wpj_gath = dram.tile([NC * WPJ_CH], mlp_dt, addr_space="Shared")

        def wprep_emit():
            # tanh + x64 + fp8-cast of own 1/8th of the MLP weights
            with tc.tile_pool(name="wprep", bufs=3) as wprep:
                for src, dst, nrb, ncols in ((wfc_ch_d, wfc_bounce, 2, FF),
                                             (wpj_ch_d, wpj_bounce, 8, D)):
                    for rb in range(nrb):
                        for ci in range(ncols // 2048):
                            off = rb * 128 * ncols + ci * 2048
                            raw = wprep.tile([128, 2048], dt.float32,
                                             tag="wraw")
                            nc.sync.dma_start(
                                out=raw[:],
                                in_=bass.AP(tensor=src.tensor,
                                            offset=src.offset + off,
                                            ap=[[ncols, 128], [1, 2048]]))
                            tnh = wprep.tile([128, 2048], dt.float32,
                                             tag="wtnh")
                            nc.scalar.activation(tnh[:], raw[:], AF.Tanh)
                            w8 = wprep.tile([128, 2048], mlp_dt, tag="w8")
                            if F8_MLP:
                                nc.vector.tensor_scalar(w8[:], tnh[:], WS,
                                                        None, op0=OP.mult)
                            else:
                                nc.vector.tensor_copy(w8[:], tnh[:])
                            nc.sync.dma_start(
                                out=bass.AP(tensor=dst.tensor,
                                            offset=dst.offset + off,
                                            ap=[[ncols, 128], [1, 2048]]),
                                in_=w8[:])

        # ---------- Phase A: x -> LN1 -> h^T ----------
        def layernorm(x_t, h_t, gk, bk):
            with tc.tile_pool(name="lnp", bufs=2) as lp:
                st = lp.tile([128, 4, 6], dt.float32, tag="st")
                xr = x_t[:].rearrange("p (n f) -> p n f", n=4)
                for sg in range(4):
                    nc.vector.bn_stats(out=st[:, sg, :], in_=xr[:, sg, :])
                mv = lp.tile([128, 2], dt.float32, tag="mv")
                nc.vector.bn_aggr(out=mv[:], in_=st[:])
                std = lp.tile([128, 1], dt.float32, tag="sd")
                nc.scalar.activation(std[:], mv[:, 1:2], AF.Sqrt,
                                     bias=eps_t[:])
                rstd = lp.tile([128, 1], dt.float32, tag="rs")
                nc.vector.reciprocal(rstd[:], std[:])
                nc.vector.tensor_scalar(h_t[:], x_t[:], mv[:, 0:1], rstd[:],
                                        op0=OP.subtract, op1=OP.mult)
                if gk in ln_bc:
                    nc.vector.tensor_mul(h_t[:], h_t[:], ln_bc[gk][:])
                if bk in ln_bc:
                    nc.vector.tensor_add(h_t[:], h_t[:], ln_bc[bk][:])

        with tc.tile_pool(name="xa", bufs=2) as xa, \
             tc.tile_pool(name="ha", bufs=2) as ha, \
             tc.tile_pool(name="trps", bufs=4, space="PSUM") as trps:
            for tb in range(4):
                x_t = xa.tile([128, D], dt.float32, tag="x")
                nc.sync.dma_start(out=x_t[:],
                                  in_=xl_d[tb * 128:(tb + 1) * 128, :])
                h_t = ha.tile([128, D], dt.float32, tag="h")
                layernorm(x_t, h_t, "g1", "b1")
                for dj in range(16):
                    ps = trps.tile([128, 128], dt.float32, tag="tp")
                    nc.tensor.transpose(ps[:], h_t[:, dj * 128:(dj + 1) * 128],
                                        ident[:])
                    nc.vector.tensor_copy(hT[:, dj, tb * 128:(tb + 1) * 128],
                                          ps[:])

        # ---------- Phase B: QKV (feature-major K/Q, token-major V) ----
        k_bounce = [dram.tile([KHALF], dt.bfloat16, name=f"kb{i}")
                    for i in range(2)]
        v_bounce = [dram.tile([VHALF], dt.bfloat16, name=f"vb{i}")
                    for i in range(2)]
        k_gath = [dram.tile([NC * KHALF], dt.bfloat16, addr_space="Shared",
                            name=f"kg{i}") for i in range(2)]
        v_gath = [dram.tile([NC * VHALF], dt.bfloat16, addr_space="Shared",
                            name=f"vg{i}") for i in range(2)]

        qkv_pool = tc.tile_pool(name="qkv", bufs=3)
        qkvp = qkv_pool.__enter__()
        qkv_ps_pool = tc.tile_pool(name="qkvps", bufs=1, space="PSUM")
        qkvps = qkv_ps_pool.__enter__()
        kacc_pool = tc.tile_pool(name="kacc", bufs=1)
        kaccp = kacc_pool.__enter__()
        kacc = kaccp.tile([128, 16, 512], dt.bfloat16, name="kacc")
        vacc = [kaccp.tile([128, D], dt.bfloat16, name=f"vacc{t}")
                for t in range(4)]

        def proj_fmajor(wT_dram, pss, dest, bank):
            # features [pss*512, pss*512+512) of w^T h^T -> dest[:, 4pss..]
            ps = [qkvps.tile([128, 512], dt.float32, tag=f"q{bank * 4 + i}",
                             name=f"ps_{wT_dram.tensor.name}_{pss}_{i}")
                  for i in range(4)]
            for dj in range(16):
                raw = qkvp.tile([128, 512], dt.float32, tag="qkraw")
                nc.sync.dma_start(
                    out=raw[:],
                    in_=wT_dram[dj * 128:(dj + 1) * 128,
                                pss * 512:(pss + 1) * 512])
                wt = qkvp.tile([128, 512], dt.bfloat16, tag="qktnh")
                nc.scalar.activation(wt[:], raw[:], AF.Tanh)
                for ft in range(4):
                    nc.tensor.matmul(ps[ft][:],
                                     wt[:, ft * 128:(ft + 1) * 128],
                                     hT[:, dj, :],
                                     start=(dj == 0), stop=(dj == 15))
            for ft in range(4):
                nc.vector.tensor_copy(dest[:, pss * 4 + ft, :], ps[ft][:])

        def proj_v(fgp, bank):
            # token-major v for features [fgp*1024, fgp*1024+1024)
            ps = [qkvps.tile([128, 512], dt.float32, tag=f"q{i}",
                             name=f"ps_v_{fgp}_{i}")
                  for i in range(8)]
            for dj in range(16):
                raw = qkvp.tile([128, 1024], dt.float32, tag="vraw")
                nc.sync.dma_start(
                    out=raw[:],
                    in_=wvT_d[dj * 128:(dj + 1) * 128,
                              fgp * 1024:(fgp + 1) * 1024])
                wt = qkvp.tile([128, 1024], dt.bfloat16, tag="vtnh")
                nc.scalar.activation(wt[:], raw[:], AF.Tanh)
                for tt in range(4):
                    for fg2 in range(2):
                        nc.tensor.matmul(
                            ps[tt * 2 + fg2][:],
                            hT[:, dj, tt * 128:(tt + 1) * 128],
                            wt[:, fg2 * 512:(fg2 + 1) * 512],
                            start=(dj == 0), stop=(dj == 15))
            for tt in range(4):
                for fg2 in range(2):
                    nc.vector.tensor_copy(
                        vacc[tt][:, fgp * 1024 + fg2 * 512:
                                 fgp * 1024 + fg2 * 512 + 512],
                        ps[tt * 2 + fg2][:])

        def dump_k(half):
            nc.sync.dma_start(
                out=bass.AP(tensor=k_bounce[half].tensor,
                            offset=k_bounce[half].offset,
                            ap=[[512, 128], [128 * 512, 8], [1, 512]]),
                in_=kacc[:, half * 8:(half + 1) * 8, :])

        def dump_v(fgp):
            for tt in range(4):
                nc.sync.dma_start(
                    out=bass.AP(tensor=v_bounce[fgp].tensor,
                                offset=v_bounce[fgp].offset + tt * 128 * 1024,
                                ap=[[1024, 128], [1, 1024]]),
                    in_=vacc[tt][:, fgp * 1024:(fgp + 1) * 1024])

        import bass_rust as _br
        _cc_prev = [None]

        def ag(src, dst):
            cc = nc.gpsimd.collective_compute(
                "AllGather", OP.bypass, replica_groups=[list(range(NC))],
                ins=[src[:]], outs=[dst[:]])
            if _cc_prev[0] is not None:
                _br.add_dep_helper(cc.ins, _cc_prev[0].ins, sync=False,
                                   reason="cc issue order")
            _cc_prev[0] = cc

        proj_fmajor(wkT_d, 0, kacc, 0)
        proj_fmajor(wkT_d, 1, kacc, 1)
        dump_k(0)
        ag(k_bounce[0], k_gath[0])
        proj_v(0, 0)
        dump_v(0)
        ag(v_bounce[0], v_gath[0])
        proj_fmajor(wkT_d, 2, kacc, 1)
        proj_fmajor(wkT_d, 3, kacc, 0)
        dump_k(1)
        ag(k_bounce[1], k_gath[1])
        proj_v(1, 1)
        dump_v(1)
        ag(v_bounce[1], v_gath[1])
        wprep_emit()
        ag(wfc_bounce, wfc_gath)
        ag(wpj_bounce, wpj_gath)
        for pss in range(4):
            proj_fmajor(wqT_d, pss, QT, pss % 2)

        kacc_pool.__exit__(None, None, None)
        qkv_ps_pool.__exit__(None, None, None)
        qkv_pool.__exit__(None, None, None)

        # ---------- Phase C: attention (causality via per-core masks) ----
        # rank r's gather block holds its 512 local tokens (256/batch,
        # rows {r, r+8, ...}); every core scans all 8 ranks per batch.
        OT = main.tile([128, 16, 512], op_dt, tag="bigA", bufs=2, name="OT")
        with tc.tile_pool(name="kvh", bufs=1) as kvh, \
             tc.tile_pool(name="att", bufs=6) as att, \
             tc.tile_pool(name="stps", bufs=3, space="PSUM") as stps, \
             tc.tile_pool(name="otps", bufs=2, space="PSUM") as otps, \
             tc.tile_pool(name="dnps", bufs=2, space="PSUM") as dnps, \
             tc.tile_pool(name="bcps", bufs=1, space="PSUM") as bcps:
            for hg in range(4):
                gh = hg // 2          # K/V gather half holding this hg
                ho = (hg % 2) * 4 * 128   # head offset inside the half
                kt_g, v_g = [], []
                for r in range(NC):
                    kt = kvh.tile([128, 4, 512], dt.bfloat16, tag="kth",
                                  bufs=11, name=f"kt_{hg}_{r}")
                    nc.sync.dma_start(
                        out=kt[:],
                        in_=bass.AP(
                            tensor=k_gath[gh].tensor,
                            offset=k_gath[gh].offset + r * KHALF
                            + ho * 512,
                            ap=[[512, 128], [128 * 512, 4], [1, 512]]))
                    kt_g.append(kt)
                    vt = kvh.tile([128, 4, 4, 128], dt.bfloat16,
                                  tag="vth", bufs=11, name=f"vt_{hg}_{r}")
                    nc.sync.dma_start(
                        out=vt[:],
                        in_=bass.AP(
                            tensor=v_gath[gh].tensor,
                            offset=v_gath[gh].offset + r * VHALF + ho,
                            ap=[[1024, 128], [128 * 1024, 4],
                                [128, 4], [1, 128]]))
                    v_g.append(vt)
                for b in range(2):
                    qoff = b * 256
                    for hh in range(4):
                        h = hg * 4 + hh
                        ot_ps = otps.tile([128, 256], dt.float32, tag="ot")
                        dn_ps = dnps.tile([1, 256], dt.float32, tag="dn")
                        for r in range(NC):
                            # st cols 0:256 = ksub0 x q[0:256],
                            #    cols 256:384 = ksub1 x q[128:256]
                            # (ksub1 is invisible to q[0:128] -> skipped)
                            st = stps.tile([128, 384], dt.float32, tag="st")
                            nc.tensor.matmul(
                                st[:, 0:256],
                                kt_g[r][:, hh, qoff:qoff + 128],
                                QT[:, h, qoff:qoff + 256],
                                start=True, stop=False,
                                skip_group_check=True)
                            nc.tensor.matmul(
                                st[:, 256:384],
                                kt_g[r][:, hh, qoff + 128:qoff + 256],
                                QT[:, h, qoff + 128:qoff + 256],
                                start=False, stop=False,
                                skip_group_check=True)
                            nc.tensor.matmul(
                                st[:], ident_b[:], masks[:, r, :],
                                start=False, stop=True,
                                skip_group_check=True)
                            pt = att.tile([128, 384], dt.bfloat16, tag="pt")
                            nc.scalar.activation(pt[:], st[:], AF.Exp)
                            last = (r == NC - 1)
                            first = (r == 0)
                            nc.tensor.matmul(
                                ot_ps[:],
                                v_g[r][:, b * 2, hh, :],
                                pt[:, 0:256],
                                start=first, stop=False,
                                skip_group_check=True)
                            nc.tensor.matmul(
                                ot_ps[:, 128:256],
                                v_g[r][:, b * 2 + 1, hh, :],
                                pt[:, 256:384],
                                start=False, stop=last,
                                skip_group_check=True)
                            nc.tensor.matmul(
                                dn_ps[:], ones_col_b[:],
                                pt[:, 0:256],
                                start=first, stop=False,
                                skip_group_check=True)
                            nc.tensor.matmul(
                                dn_ps[:, 128:256], ones_col_b[:],
                                pt[:, 256:384],
                                start=False, stop=last,
                                skip_group_check=True)
                        dn_sb = att.tile([1, 256], dt.float32, tag="dns")
                        nc.vector.tensor_copy(dn_sb[:], dn_ps[:])
                        bc_ps = bcps.tile([128, 256], dt.float32, tag="bc")
                        nc.tensor.matmul(bc_ps[:], ones_row[:], dn_sb[:],
                                         start=True, stop=True)
                        rec_sb = att.tile([128, 256], dt.float32, tag="bcs")
                        nc.vector.reciprocal(rec_sb[:], bc_ps[:])
                        nc.vector.tensor_mul(OT[:, h, qoff:qoff + 256],
                                             ot_ps[:], rec_sb[:])

        # ---------- Phase D: out-proj + residual -> h2; LN2 -> m^T ----
        h2_pool = tc.tile_pool(name="h2a", bufs=1)
        h2a = h2_pool.__enter__()
        h2acc = [h2a.tile([128, D], dt.float32, name=f"h2_{t}")
                 for t in range(4)]
        KS = 2 if F8_OP else 1
        with tc.tile_pool(name="wo", bufs=3) as wop, \
             tc.tile_pool(name="xd", bufs=3) as xd, \
             tc.tile_pool(name="dps", bufs=1, space="PSUM") as dps:
            for dgp in range(2):
                ps = [dps.tile([128, 512], dt.float32, tag=f"d{i}",
                               name=f"dp_{dgp}_{i}") for i in range(8)]
                for og in range(16 // KS):
                    raw = wop.tile([128, KS, 1024], dt.float32, tag="oraw")
                    nc.sync.dma_start(
                        out=raw[:],
                        in_=bass.AP(tensor=woT_d.tensor,
                                    offset=woT_d.offset
                                    + og * KS * 128 * D + dgp * 1024,
                                    ap=[[D, 128], [128 * D, KS], [1, 1024]]))
                    if F8_OP:
                        tnh = wop.tile([128, KS, 1024], dt.float32,
                                       tag="otnh32")
                        nc.scalar.activation(tnh[:], raw[:], AF.Tanh)
                        wt = wop.tile([128, KS, 1024], op_dt, tag="otnh")
                        nc.vector.tensor_scalar(wt[:], tnh[:], WS, None,
                                                op0=OP.mult)
                    else:
                        wt = wop.tile([128, KS, 1024], dt.bfloat16,
                                      tag="otnh")
                        nc.scalar.activation(wt[:], raw[:], AF.Tanh)
                    for tt in range(4):
                        for dg2 in range(2):
                            if F8_OP:
                                nc.tensor.matmul(
                                    ps[tt * 2 + dg2][:],
                                    OT[:, og * 2:og * 2 + 2,
                                       tt * 128:(tt + 1) * 128],
                                    wt[:, :, dg2 * 512:(dg2 + 1) * 512],
                                    start=(og == 0), stop=(og == 7),
                                    perf_mode=DR)
                            else:
                                nc.tensor.matmul(
                                    ps[tt * 2 + dg2][:],
                                    OT[:, og, tt * 128:(tt + 1) * 128],
                                    wt[:, 0, dg2 * 512:(dg2 + 1) * 512],
                                    start=(og == 0), stop=(og == 15))
                for tt in range(4):
                    x_t = xd.tile([128, 1024], dt.float32, tag="x2")
                    nc.sync.dma_start(
                        out=x_t[:],
                        in_=xl_d[tt * 128:(tt + 1) * 128,
                                 dgp * 1024:(dgp + 1) * 1024])
                    for dg2 in range(2):
                        sl = slice(dgp * 1024 + dg2 * 512,
                                   dgp * 1024 + dg2 * 512 + 512)
                        if F8_OP:
                            nc.vector.tensor_scalar(
                                h2acc[tt][:, sl], ps[tt * 2 + dg2][:],
                                1.0 / WS, None, op0=OP.mult)
                            nc.vector.tensor_add(h2acc[tt][:, sl],
                                                 h2acc[tt][:, sl],
                                                 bo_bc[:, sl])
                        else:
                            nc.vector.tensor_add(h2acc[tt][:, sl],
                                                 ps[tt * 2 + dg2][:],
                                                 bo_bc[:, sl])
                        nc.vector.tensor_add(
                            h2acc[tt][:, sl], h2acc[tt][:, sl],
                            x_t[:, dg2 * 512:(dg2 + 1) * 512])

        mT = main.tile([128, 16, 512], mlp_dt, tag="bigA", bufs=2, name="mT")
        with tc.tile_pool(name="md", bufs=2) as md, \
             tc.tile_pool(name="trps2", bufs=4, space="PSUM") as trps2:
            for tb in range(4):
                m_t = md.tile([128, D], dt.float32, tag="m")
                layernorm(h2acc[tb], m_t, "g2", "b2")
                for dj in range(16):
                    ps = trps2.tile([128, 128], dt.float32, tag="tp2")
                    nc.tensor.transpose(ps[:], m_t[:, dj * 128:(dj + 1) * 128],
                                        ident[:])
                    nc.vector.tensor_copy(mT[:, dj, tb * 128:(tb + 1) * 128],
                                          ps[:])

        # ---------- Phase E: MLP (fp8 DoubleRow) ----------
        gt_pool = tc.tile_pool(name="gtpl", bufs=1)
        gtpl = gt_pool.__enter__()
        GT = gtpl.tile([128, 64, 512], mlp_dt, name="GT")
        wfcT_v = wfc_gath
        wpjT_v = wpj_gath
        MKS = 2 if F8_MLP else 1

        with tc.tile_pool(name="wfc", bufs=6) as wfcp, \
             tc.tile_pool(name="ups", bufs=2, space="PSUM") as ups:
            for grp in range(16):        # 4 f-tiles (512 features) per group
                ps = [ups.tile([128, 512], dt.float32, tag=f"u{i}",
                               name=f"u_{grp}_{i}") for i in range(4)]
                for djp in range(16 // MKS):
                    w2 = wfcp.tile([128, MKS, 512], mlp_dt, tag="wfct")
                    nc.sync.dma_start(
                        out=w2[:],
                        in_=bass.AP(tensor=wfcT_v.tensor,
                                    offset=wfcT_v.offset
                                    + djp * MKS * 128 * FF + grp * 512,
                                    ap=[[FF, 128], [128 * FF, MKS],
                                        [1, 512]]))
                    for f4 in range(4):
                        if F8_MLP:
                            nc.tensor.matmul(
                                ps[f4][:],
                                w2[:, :, f4 * 128:(f4 + 1) * 128],
                                mT[:, djp * 2:djp * 2 + 2, :],
                                start=(djp == 0), stop=(djp == 7),
                                perf_mode=DR)
                        else:
                            nc.tensor.matmul(
                                ps[f4][:],
                                w2[:, 0, f4 * 128:(f4 + 1) * 128],
                                mT[:, djp, :],
                                start=(djp == 0), stop=(djp == 15))
                for f4 in range(4):
                    fti = grp * 4 + f4
                    nc.scalar.activation(GT[:, fti, :], ps[f4][:],
                                         AF.Gelu_apprx_tanh,
                                         bias=bfc_pp[:, fti:fti + 1],
                                         scale=(1.0 / WS if F8_MLP else 1.0))

        with tc.tile_pool(name="wpj", bufs=4) as wpjp, \
             tc.tile_pool(name="yps", bufs=1, space="PSUM") as yps, \
             tc.tile_pool(name="outp", bufs=4) as outp:
            for ttp in range(2):
                ps = [yps.tile([128, 512], dt.float32, tag=f"y{i}",
                               name=f"y_{ttp}_{i}") for i in range(8)]
                for fp in range(64 // MKS):
                    wp2 = wpjp.tile([128, MKS, 2048], mlp_dt, tag="wpjt")
                    nc.sync.dma_start(
                        out=wp2[:],
                        in_=bass.AP(tensor=wpjT_v.tensor,
                                    offset=wpjT_v.offset
                                    + fp * MKS * 128 * D,
                                    ap=[[D, 128], [128 * D, MKS],
                                        [1, 2048]]))
                    for tt2 in range(2):
                        tt = ttp * 2 + tt2
                        for dg in range(4):
                            if F8_MLP:
                                nc.tensor.matmul(
                                    ps[tt2 * 4 + dg][:],
                                    GT[:, fp * 2:fp * 2 + 2,
                                       tt * 128:(tt + 1) * 128],
                                    wp2[:, :, dg * 512:(dg + 1) * 512],
                                    start=(fp == 0), stop=(fp == 31),
                                    perf_mode=DR)
                            else:
                                nc.tensor.matmul(
                                    ps[tt2 * 4 + dg][:],
                                    GT[:, fp, tt * 128:(tt + 1) * 128],
                                    wp2[:, 0, dg * 512:(dg + 1) * 512],
                                    start=(fp == 0), stop=(fp == 63))
                for tt2 in range(2):
                    tt = ttp * 2 + tt2
                    for dg in range(4):
                        sl = slice(dg * 512, dg * 512 + 512)
                        o_t = outp.tile([128, 512], dt.float32, tag="o")
                        if F8_MLP:
                            nc.vector.tensor_scalar(
                                o_t[:], ps[tt2 * 4 + dg][:], 1.0 / WS, None,
                                op0=OP.mult)
                            nc.vector.tensor_add(o_t[:], o_t[:],
                                                 bpj_bc[:, sl])
                        else:
                            nc.vector.tensor_add(o_t[:], ps[tt2 * 4 + dg][:],
                                                 bpj_bc[:, sl])
                        nc.vector.tensor_add(o_t[:], o_t[:],
                                             h2acc[tt][:, sl])
                        nc.sync.dma_start(
                            out=out_d[tt * 128:(tt + 1) * 128, sl],
                            in_=o_t[:])
        gt_pool.__exit__(None, None, None)
        h2_pool.__exit__(None, None, None)
        stack.close()

    nc.compile()
    return nc


def _host_prep(inputs):
    f32 = lambda k: np.ascontiguousarray(np.asarray(inputs[k], np.float32))
    x = f32("hidden_states")
    wqT = np.ascontiguousarray(f32("wq").T)
    wkT = np.ascontiguousarray(f32("wk").T)
    wvT = np.ascontiguousarray(f32("wv").T)
    woT = np.ascontiguousarray(f32("wo").T)
    wfcT = np.ascontiguousarray(f32("w_fc").T).ravel()
    wpjT = np.ascontiguousarray(f32("w_proj").T).ravel()
    # causal masks per core: q token = 8*qf + c, k token = 8*(ks*128+kp) + r
    # packed [128, 8, 384]: cols 0:256 = ksub0 x q[0:256],
    #                       cols 256:384 = ksub1 x q[128:256]
    kp = np.arange(128)[:, None, None]
    rr = np.arange(8)[None, :, None]
    ks = np.concatenate([np.zeros(256, np.int64),
                         np.ones(128, np.int64)])[None, None, :]
    qf = np.concatenate([np.arange(256),
                         np.arange(128, 256)])[None, None, :]
    in_maps = []
    for c in range(NC):
        mask = np.where(8 * (ks * 128 + kp) + rr <= 8 * qf + c,
                        0.0, -1e9).astype(np.float32)
        mask = mask.astype(ml_dtypes.bfloat16)
        in_maps.append({
            "xl": np.concatenate([x[0, c::NC, :], x[1, c::NC, :]], 0),
            "wqT": wqT, "wkT": wkT, "wvT": wvT, "woT": woT,
            "wfc_ch": wfcT[c * WFC_CH:(c + 1) * WFC_CH],
            "wpj_ch": wpjT[c * WPJ_CH:(c + 1) * WPJ_CH],
            "mask": mask,
            "ln1g": f32("ln1_g"), "ln1b": f32("ln1_b"),
            "ln2g": f32("ln2_g"), "ln2b": f32("ln2_b"),
            "bo": f32("bo"), "bfc": f32("b_fc"), "bpj": f32("b_proj"),
        })
    return in_maps


def kernel(**inputs) -> np.ndarray:
    in_maps = _host_prep(inputs)
    key = (not bool(np.all(np.asarray(inputs["ln1_g"]) == 1.0)),
           not bool(np.all(np.asarray(inputs["ln1_b"]) == 0.0)),
           not bool(np.all(np.asarray(inputs["ln2_g"]) == 1.0)),
           not bool(np.all(np.asarray(inputs["ln2_b"]) == 0.0)))
    if key not in _CACHE:
        _CACHE[key] = _build(*key)
    nc = _CACHE[key]
    res = run_bass_kernel_spmd(nc, in_maps, core_ids=list(range(NC)))
    if res.exec_time_ns is not None:
        print(f"HW exec time: {res.exec_time_ns} ns")
    out = np.zeros((B, S, D), np.float32)
    for c in range(NC):
        o = res.results[c]["out"]
        out[0, c::NC] = o[:CH]
        out[1, c::NC] = o[CH:]
    return out


# revision 30
# speedup vs baseline: 1.4663x; 1.1244x over previous
"""BinaryGPTNeoBlock on 8 trn2 NeuronCores.

Sequence-parallel over 8 cores: core c owns rows {c, c+8, ...} of both
batches (256 per batch, 512 total); causality is per-core mask data so
the program stays SPMD-uniform. K/V are projected feature-/token-major
directly (no PE transposes), cast bf16, and AllGathered in two halves
each (interleaved with the projection passes) so attention starts with
no stall. MLP weights: each core tanh's + scales (x64) its 1/8th into
fp8, two AllGathers share them, and fc/proj run fp8 DoubleRow matmuls
(2x PE rate); the 1/64 descale folds into PSUM evacuation.

Self-contained: hardcodes shapes; host only shards/transposes/builds masks.
"""

import numpy as np
import ml_dtypes

import concourse.bass as bass
import concourse.tile as tile
from concourse import bacc, mybir
from concourse.bass_utils import run_bass_kernel_spmd
from concourse.masks import make_identity

B, S, D = 2, 2048, 2048
H = 16
HD = 128
FF = 4 * D
EPS = 1e-5
NC = 8
CH = 256               # q-chunk length (S // NC)
TL = 2 * CH            # 512 local rows (one chunk per batch)
WFC_CH = 256 * FF      # own d-rows of wfcT
WPJ_CH = 1024 * D      # own f-rows of wpjT
WS = 64.0              # fp8 weight pre-scale (undone at PSUM evacuation)

F8_MLP = True          # fc/proj in fp8 DoubleRow
F8_OP = False          # out-proj in fp8 DoubleRow

dt = mybir.dt
AF = mybir.ActivationFunctionType
OP = mybir.AluOpType
DR = mybir.MatmulPerfMode.DoubleRow

_CACHE = {}


def _build(apply_g1, apply_b1, apply_g2, apply_b2):
    nc = bacc.Bacc("TRN2", target_bir_lowering=False, debug=False,
                   num_devices=NC)

    xl_d = nc.dram_tensor("xl", [TL, D], dt.float32, kind="ExternalInput").ap()
    wqT_d = nc.dram_tensor("wqT", [D, D], dt.float32, kind="ExternalInput").ap()
    wkT_d = nc.dram_tensor("wkT", [D, D], dt.float32, kind="ExternalInput").ap()
    wvT_d = nc.dram_tensor("wvT", [D, D], dt.float32, kind="ExternalInput").ap()
    woT_d = nc.dram_tensor("woT", [D, D], dt.float32, kind="ExternalInput").ap()
    wfc_ch_d = nc.dram_tensor("wfc_ch", [WFC_CH], dt.float32,
                              kind="ExternalInput").ap()
    wpj_ch_d = nc.dram_tensor("wpj_ch", [WPJ_CH], dt.float32,
                              kind="ExternalInput").ap()
    mask_d = nc.dram_tensor("mask", [128, 8, 384], dt.bfloat16,
                            kind="ExternalInput").ap()
    ln1g_d = nc.dram_tensor("ln1g", [D], dt.float32, kind="ExternalInput").ap()
    ln1b_d = nc.dram_tensor("ln1b", [D], dt.float32, kind="ExternalInput").ap()
    ln2g_d = nc.dram_tensor("ln2g", [D], dt.float32, kind="ExternalInput").ap()
    ln2b_d = nc.dram_tensor("ln2b", [D], dt.float32, kind="ExternalInput").ap()
    bo_d = nc.dram_tensor("bo", [D], dt.float32, kind="ExternalInput").ap()
    bfc_d = nc.dram_tensor("bfc", [FF], dt.float32, kind="ExternalInput").ap()
    bpj_d = nc.dram_tensor("bpj", [D], dt.float32, kind="ExternalInput").ap()
    out_d = nc.dram_tensor("out", [TL, D], dt.float32,
                           kind="ExternalOutput").ap()

    mlp_dt = dt.float8e4 if F8_MLP else dt.bfloat16
    op_dt = dt.float8e4 if F8_OP else dt.bfloat16
    KHALF = 1024 * TL        # elems of one kT half per core
    VHALF = TL * 1024

    def bcast_row(src_ap, n):
        return bass.AP(tensor=src_ap.tensor, offset=src_ap.offset,
                       ap=[[0, 128], [1, n]])

    with tile.TileContext(nc) as tc:
        import contextlib
        stack = contextlib.ExitStack()
        main = stack.enter_context(tc.tile_pool(name="main", bufs=1))
        dram = stack.enter_context(
            tc.tile_pool(name="dram", bufs=1, space="DRAM"))

        ident = main.tile([128, 128], dt.float32)
        make_identity(nc, ident[:])
        ident_b = main.tile([128, 128], dt.bfloat16)
        nc.vector.tensor_copy(ident_b[:], ident[:])
        ones_col_b = main.tile([128, 1], dt.bfloat16)
        nc.vector.memset(ones_col_b[:], 1.0)
        ones_row = main.tile([1, 128], dt.float32)
        nc.vector.memset(ones_row[:], 1.0)
        eps_t = main.tile([128, 1], dt.float32)
        nc.vector.memset(eps_t[:], EPS)
        bo_bc = main.tile([128, D], dt.float32)
        nc.sync.dma_start(out=bo_bc[:], in_=bcast_row(bo_d, D))
        bpj_bc = main.tile([128, D], dt.float32)
        nc.sync.dma_start(out=bpj_bc[:], in_=bcast_row(bpj_d, D))
        masks = main.tile([128, 8, 384], dt.bfloat16)
        nc.sync.dma_start(out=masks[:], in_=mask_d[:])
        bfc_pp = main.tile([128, FF // 128], dt.float32)
        nc.sync.dma_start(
            out=bfc_pp[:],
            in_=bass.AP(tensor=bfc_d.tensor, offset=bfc_d.offset,
                        ap=[[1, 128], [128, FF // 128]]))
        ln_bc = {}
        for nm, flag, src in (("g1", apply_g1, ln1g_d),
                              ("b1", apply_b1, ln1b_d),
                              ("g2", apply_g2, ln2g_d),
                              ("b2", apply_b2, ln2b_d)):
            if flag:
                t = main.tile([128, D], dt.float32, name=f"ln_{nm}")
                nc.sync.dma_start(out=t[:], in_=bcast_row(src, D))
                ln_bc[nm] = t

        # rotating big activation slots (16KB/part each, 2 slots)
        hT = main.tile([128, 16, 512], dt.bfloat16, tag="bigA", bufs=2,
                       name="hT")
        QT = main.tile([128, 16, 512], dt.bfloat16, tag="bigA", bufs=2,
                       name="QT")

        wfc_bounce = dram.tile([WFC_CH], mlp_dt)
        wpj_bounce = dram.tile([WPJ_CH], mlp_dt)
        wfc_gath = dram.tile([NC * WFC_CH], mlp_dt, addr_space="Shared")
        wpj_gath = dram.tile([NC * WPJ_CH], mlp_dt, addr_space="Shared")

        def wprep_emit():
            # tanh + x64 + fp8-cast of own 1/8th of the MLP weights
            with tc.tile_pool(name="wprep", bufs=3) as wprep:
                for src, dst, nrb, ncols in ((wfc_ch_d, wfc_bounce, 2, FF),
                                             (wpj_ch_d, wpj_bounce, 8, D)):
                    for rb in range(nrb):
                        for ci in range(ncols // 2048):
                            off = rb * 128 * ncols + ci * 2048
                            raw = wprep.tile([128, 2048], dt.float32,
                                             tag="wraw")
                            nc.sync.dma_start(
                                out=raw[:],
                                in_=bass.AP(tensor=src.tensor,
                                            offset=src.offset + off,
                                            ap=[[ncols, 128], [1, 2048]]))
                            tnh = wprep.tile([128, 2048], dt.float32,
                                             tag="wtnh")
                            nc.scalar.activation(tnh[:], raw[:], AF.Tanh)
                            w8 = wprep.tile([128, 2048], mlp_dt, tag="w8")
                            if F8_MLP:
                                nc.vector.tensor_scalar(w8[:], tnh[:], WS,
                                                        None, op0=OP.mult)
                            else:
                                nc.vector.tensor_copy(w8[:], tnh[:])
                            nc.sync.dma_start(
                                out=bass.AP(tensor=dst.tensor,
                                            offset=dst.offset + off,
                                            ap=[[ncols, 128], [1, 2048]]),
                                in_=w8[:])

        # ---------- Phase A: x -> LN1 -> h^T ----------
        def layernorm(x_t, h_t, gk, bk):
            with tc.tile_pool(name="lnp", bufs=2) as lp:
                st = lp.tile([128, 4, 6], dt.float32, tag="st")
                xr = x_t[:].rearrange("p (n f) -> p n f", n=4)
                for sg in range(4):
                    nc.vector.bn_stats(out=st[:, sg, :], in_=xr[:, sg, :])
                mv = lp.tile([128, 2], dt.float32, tag="mv")
                nc.vector.bn_aggr(out=mv[:], in_=st[:])
                std = lp.tile([128, 1], dt.float32, tag="sd")
                nc.scalar.activation(std[:], mv[:, 1:2], AF.Sqrt,
                                     bias=eps_t[:])
                rstd = lp.tile([128, 1], dt.float32, tag="rs")
                nc.vector.reciprocal(rstd[:], std[:])
                nc.vector.tensor_scalar(h_t[:], x_t[:], mv[:, 0:1], rstd[:],
                                        op0=OP.subtract, op1=OP.mult)
                if gk in ln_bc:
                    nc.vector.tensor_mul(h_t[:], h_t[:], ln_bc[gk][:])
                if bk in ln_bc:
                    nc.vector.tensor_add(h_t[:], h_t[:], ln_bc[bk][:])

        with tc.tile_pool(name="xa", bufs=2) as xa, \
             tc.tile_pool(name="ha", bufs=1) as ha, \
             tc.tile_pool(name="trps", bufs=4, space="PSUM") as trps:
            h_ts = []
            for tb in range(4):
                x_t = xa.tile([128, D], dt.float32, tag="x")
                nc.sync.dma_start(out=x_t[:],
                                  in_=xl_d[tb * 128:(tb + 1) * 128, :])
                h_t = ha.tile([128, D], dt.float32, name=f"h_{tb}")
                layernorm(x_t, h_t, "g1", "b1")
                h_ts.append(h_t)
            # dj-major so hT[:, dj, :] completes early -> K matmuls overlap
            for dj in range(16):
                for tb in range(4):
                    ps = trps.tile([128, 128], dt.float32, tag="tp")
                    nc.tensor.transpose(
                        ps[:], h_ts[tb][:, dj * 128:(dj + 1) * 128],
                        ident[:])
                    nc.vector.tensor_copy(hT[:, dj, tb * 128:(tb + 1) * 128],
                                          ps[:])

        # ---------- Phase B: QKV (feature-major K/Q, token-major V) ----
        k_bounce = [dram.tile([KHALF], dt.bfloat16, name=f"kb{i}")
                    for i in range(2)]
        v_bounce = [dram.tile([VHALF], dt.bfloat16, name=f"vb{i}")
                    for i in range(2)]
        k_gath = [dram.tile([NC * KHALF], dt.bfloat16, addr_space="Shared",
                            name=f"kg{i}") for i in range(2)]
        v_gath = [dram.tile([NC * VHALF], dt.bfloat16, addr_space="Shared",
                            name=f"vg{i}") for i in range(2)]

        qkv_pool = tc.tile_pool(name="qkv", bufs=3)
        qkvp = qkv_pool.__enter__()
        qkv_ps_pool = tc.tile_pool(name="qkvps", bufs=1, space="PSUM")
        qkvps = qkv_ps_pool.__enter__()
        kacc_pool = tc.tile_pool(name="kacc", bufs=1)
        kaccp = kacc_pool.__enter__()
        kacc = kaccp.tile([128, 16, 512], dt.bfloat16, name="kacc")
        vacc = [kaccp.tile([128, D], dt.bfloat16, name=f"vacc{t}")
                for t in range(4)]

        def proj_fmajor(wT_dram, pss, dest):
            # features [pss*1024, pss*1024+1024) of w^T h^T -> dest[:, 8pss..]
            ps = [qkvps.tile([128, 512], dt.float32, tag=f"q{i}",
                             name=f"ps_{wT_dram.tensor.name}_{pss}_{i}")
                  for i in range(8)]
            for dj in range(16):
                raw = qkvp.tile([128, 1024], dt.float32, tag="qkraw", bufs=6)
                nc.sync.dma_start(
                    out=raw[:],
                    in_=wT_dram[dj * 128:(dj + 1) * 128,
                                pss * 1024:(pss + 1) * 1024])
                wt = qkvp.tile([128, 1024], dt.bfloat16, tag="qktnh", bufs=4)
                nc.scalar.activation(wt[:], raw[:], AF.Tanh)
                for ft in range(8):
                    nc.tensor.matmul(ps[ft][:],
                                     wt[:, ft * 128:(ft + 1) * 128],
                                     hT[:, dj, :],
                                     start=(dj == 0), stop=(dj == 15))
            for ft in range(8):
                nc.vector.tensor_copy(dest[:, pss * 8 + ft, :], ps[ft][:])

        def proj_v(fgp):
            # token-major v for features [fgp*1024, fgp*1024+1024)
            ps = [qkvps.tile([128, 512], dt.float32, tag=f"q{i}",
                             name=f"ps_v_{fgp}_{i}")
                  for i in range(8)]
            for dj in range(16):
                raw = qkvp.tile([128, 1024], dt.float32, tag="qkraw", bufs=6)
                nc.sync.dma_start(
                    out=raw[:],
                    in_=wvT_d[dj * 128:(dj + 1) * 128,
                              fgp * 1024:(fgp + 1) * 1024])
                wt = qkvp.tile([128, 1024], dt.bfloat16, tag="qktnh", bufs=4)
                nc.scalar.activation(wt[:], raw[:], AF.Tanh)
                for tt in range(4):
                    for fg2 in range(2):
                        nc.tensor.matmul(
                            ps[tt * 2 + fg2][:],
                            hT[:, dj, tt * 128:(tt + 1) * 128],
                            wt[:, fg2 * 512:(fg2 + 1) * 512],
                            start=(dj == 0), stop=(dj == 15))
            for tt in range(4):
                for fg2 in range(2):
                    nc.vector.tensor_copy(
                        vacc[tt][:, fgp * 1024 + fg2 * 512:
                                 fgp * 1024 + fg2 * 512 + 512],
                        ps[tt * 2 + fg2][:])

        def dump_k(half):
            nc.sync.dma_start(
                out=bass.AP(tensor=k_bounce[half].tensor,
                            offset=k_bounce[half].offset,
                            ap=[[512, 128], [128 * 512, 8], [1, 512]]),
                in_=kacc[:, half * 8:(half + 1) * 8, :])

        def dump_v(fgp):
            for tt in range(4):
                nc.sync.dma_start(
                    out=bass.AP(tensor=v_bounce[fgp].tensor,
                                offset=v_bounce[fgp].offset + tt * 128 * 1024,
                                ap=[[1024, 128], [1, 1024]]),
                    in_=vacc[tt][:, fgp * 1024:(fgp + 1) * 1024])

        import bass_rust as _br
        _cc_prev = [None]

        def ag(src, dst):
            cc = nc.gpsimd.collective_compute(
                "AllGather", OP.bypass, replica_groups=[list(range(NC))],
                ins=[src[:]], outs=[dst[:]])
            if _cc_prev[0] is not None:
                _br.add_dep_helper(cc.ins, _cc_prev[0].ins, sync=False,
                                   reason="cc issue order")
            _cc_prev[0] = cc

        proj_fmajor(wkT_d, 0, kacc)
        dump_k(0)
        ag(k_bounce[0], k_gath[0])
        proj_v(0)
        dump_v(0)
        ag(v_bounce[0], v_gath[0])
        proj_fmajor(wkT_d, 1, kacc)
        dump_k(1)
        ag(k_bounce[1], k_gath[1])
        proj_v(1)
        dump_v(1)
        ag(v_bounce[1], v_gath[1])
        wprep_emit()
        ag(wfc_bounce, wfc_gath)
        ag(wpj_bounce, wpj_gath)
        proj_fmajor(wqT_d, 0, QT)
        proj_fmajor(wqT_d, 1, QT)

        kacc_pool.__exit__(None, None, None)
        qkv_ps_pool.__exit__(None, None, None)
        qkv_pool.__exit__(None, None, None)

        # ---------- Phase C: attention (causality via per-core masks) ----
        # rank r's gather block holds its 512 local tokens (256/batch,
        # rows {r, r+8, ...}); every core scans all 8 ranks per batch.
        OT = main.tile([128, 16, 512], op_dt, tag="bigA", bufs=2, name="OT")
        with tc.tile_pool(name="kvh", bufs=1) as kvh, \
             tc.tile_pool(name="att", bufs=6) as att, \
             tc.tile_pool(name="stps", bufs=3, space="PSUM") as stps, \
             tc.tile_pool(name="otps", bufs=2, space="PSUM") as otps, \
             tc.tile_pool(name="dnps", bufs=2, space="PSUM") as dnps, \
             tc.tile_pool(name="bcps", bufs=1, space="PSUM") as bcps:
            for hg in range(4):
                gh = hg // 2          # K/V gather half holding this hg
                ho = (hg % 2) * 4 * 128   # head offset inside the half
                kt_g, v_g = [], []
                for r in range(NC):
                    kt = kvh.tile([128, 4, 512], dt.bfloat16, tag="kth",
                                  bufs=11, name=f"kt_{hg}_{r}")
                    nc.sync.dma_start(
                        out=kt[:],
                        in_=bass.AP(
                            tensor=k_gath[gh].tensor,
                            offset=k_gath[gh].offset + r * KHALF
                            + ho * 512,
                            ap=[[512, 128], [128 * 512, 4], [1, 512]]))
                    kt_g.append(kt)
                    vt = kvh.tile([128, 4, 512], dt.bfloat16,
                                  tag="vth", bufs=11, name=f"vt_{hg}_{r}")
                    nc.sync.dma_start(
                        out=vt[:],
                        in_=bass.AP(
                            tensor=v_gath[gh].tensor,
                            offset=v_gath[gh].offset + r * VHALF + ho,
                            ap=[[1024, 128], [128 * 1024, 4], [1, 512]]))
                    v_g.append(vt)
                for b in range(2):
                    qoff = b * 256
                    for hh in range(4):
                        h = hg * 4 + hh
                        ot_ps = otps.tile([128, 256], dt.float32, tag="ot")
                        dn_ps = dnps.tile([1, 256], dt.float32, tag="dn")
                        for r in range(NC):
                            # st cols 0:256 = ksub0 x q[0:256],
                            #    cols 256:384 = ksub1 x q[128:256]
                            # (ksub1 is invisible to q[0:128] -> skipped)
                            st = stps.tile([128, 384], dt.float32, tag="st")
                            nc.tensor.matmul(
                                st[:, 0:256],
                                kt_g[r][:, hh, qoff:qoff + 128],
                                QT[:, h, qoff:qoff + 256],
                                start=True, stop=False,
                                skip_group_check=True)
                            nc.tensor.matmul(
                                st[:, 256:384],
                                kt_g[r][:, hh, qoff + 128:qoff + 256],
                                QT[:, h, qoff + 128:qoff + 256],
                                start=False, stop=False,
                                skip_group_check=True)
                            nc.tensor.matmul(
                                st[:], ident_b[:], masks[:, r, :],
                                start=False, stop=True,
                                skip_group_check=True)
                            pt = att.tile([128, 384], dt.bfloat16, tag="pt")
                            nc.scalar.activation(pt[:], st[:], AF.Exp)
                            last = (r == NC - 1)
                            first = (r == 0)
                            nc.tensor.matmul(
                                ot_ps[:],
                                v_g[r][:, b * 2, hh * 128:(hh + 1) * 128],
                                pt[:, 0:256],
                                start=first, stop=False,
                                skip_group_check=True)
                            nc.tensor.matmul(
                                ot_ps[:, 128:256],
                                v_g[r][:, b * 2 + 1,
                                       hh * 128:(hh + 1) * 128],
                                pt[:, 256:384],
                                start=False, stop=last,
                                skip_group_check=True)
                            nc.tensor.matmul(
                                dn_ps[:], ones_col_b[:],
                                pt[:, 0:256],
                                start=first, stop=False,
                                skip_group_check=True)
                            nc.tensor.matmul(
                                dn_ps[:, 128:256], ones_col_b[:],
                                pt[:, 256:384],
                                start=False, stop=last,
                                skip_group_check=True)
                        dn_sb = att.tile([1, 256], dt.float32, tag="dns")
                        nc.vector.tensor_copy(dn_sb[:], dn_ps[:])
                        bc_ps = bcps.tile([128, 256], dt.float32, tag="bc")
                        nc.tensor.matmul(bc_ps[:], ones_row[:], dn_sb[:],
                                         start=True, stop=True)
                        rec_sb = att.tile([128, 256], dt.float32, tag="bcs")
                        nc.vector.reciprocal(rec_sb[:], bc_ps[:])
                        nc.vector.tensor_mul(OT[:, h, qoff:qoff + 256],
                                             ot_ps[:], rec_sb[:])

        # ---------- Phase D: out-proj + residual -> h2; LN2 -> m^T ----
        h2_pool = tc.tile_pool(name="h2a", bufs=1)
        h2a = h2_pool.__enter__()
        h2acc = [h2a.tile([128, D], dt.float32, name=f"h2_{t}")
                 for t in range(4)]
        KS = 2 if F8_OP else 1
        with tc.tile_pool(name="wo", bufs=3) as wop, \
             tc.tile_pool(name="xd", bufs=3) as xd, \
             tc.tile_pool(name="dps", bufs=1, space="PSUM") as dps:
            for dgp in range(2):
                ps = [dps.tile([128, 512], dt.float32, tag=f"d{i}",
                               name=f"dp_{dgp}_{i}") for i in range(8)]
                for og in range(16 // KS):
                    raw = wop.tile([128, KS, 1024], dt.float32, tag="oraw")
                    nc.sync.dma_start(
                        out=raw[:],
                        in_=bass.AP(tensor=woT_d.tensor,
                                    offset=woT_d.offset
                                    + og * KS * 128 * D + dgp * 1024,
                                    ap=[[D, 128], [128 * D, KS], [1, 1024]]))
                    if F8_OP:
                        tnh = wop.tile([128, KS, 1024], dt.float32,
                                       tag="otnh32")
                        nc.scalar.activation(tnh[:], raw[:], AF.Tanh)
                        wt = wop.tile([128, KS, 1024], op_dt, tag="otnh")
                        nc.vector.tensor_scalar(wt[:], tnh[:], WS, None,
                                                op0=OP.mult)
                    else:
                        wt = wop.tile([128, KS, 1024], dt.bfloat16,
                                      tag="otnh")
                        nc.scalar.activation(wt[:], raw[:], AF.Tanh)
                    for tt in range(4):
                        for dg2 in range(2):
                            if F8_OP:
                                nc.tensor.matmul(
                                    ps[tt * 2 + dg2][:],
                                    OT[:, og * 2:og * 2 + 2,
                                       tt * 128:(tt + 1) * 128],
                                    wt[:, :, dg2 * 512:(dg2 + 1) * 512],
                                    start=(og == 0), stop=(og == 7),
                                    perf_mode=DR)
                            else:
                                nc.tensor.matmul(
                                    ps[tt * 2 + dg2][:],
                                    OT[:, og, tt * 128:(tt + 1) * 128],
                                    wt[:, 0, dg2 * 512:(dg2 + 1) * 512],
                                    start=(og == 0), stop=(og == 15))
                for tt in range(4):
                    x_t = xd.tile([128, 1024], dt.float32, tag="x2")
                    nc.sync.dma_start(
                        out=x_t[:],
                        in_=xl_d[tt * 128:(tt + 1) * 128,
                                 dgp * 1024:(dgp + 1) * 1024])
                    for dg2 in range(2):
                        sl = slice(dgp * 1024 + dg2 * 512,
                                   dgp * 1024 + dg2 * 512 + 512)
                        if F8_OP:
                            nc.vector.tensor_scalar(
                                h2acc[tt][:, sl], ps[tt * 2 + dg2][:],
                                1.0 / WS, None, op0=OP.mult)
                            nc.vector.tensor_add(h2acc[tt][:, sl],
                                                 h2acc[tt][:, sl],
                                                 bo_bc[:, sl])
                        else:
                            nc.vector.tensor_add(h2acc[tt][:, sl],
                                                 ps[tt * 2 + dg2][:],
                                                 bo_bc[:, sl])
                        nc.vector.tensor_add(
                            h2acc[tt][:, sl], h2acc[tt][:, sl],
                            x_t[:, dg2 * 512:(dg2 + 1) * 512])

        mT = main.tile([128, 16, 512], mlp_dt, tag="bigA", bufs=2, name="mT")
        with tc.tile_pool(name="md", bufs=2) as md, \
             tc.tile_pool(name="trps2", bufs=4, space="PSUM") as trps2:
            for tb in range(4):
                m_t = md.tile([128, D], dt.float32, tag="m")
                layernorm(h2acc[tb], m_t, "g2", "b2")
                for dj in range(16):
                    ps = trps2.tile([128, 128], dt.float32, tag="tp2")
                    nc.tensor.transpose(ps[:], m_t[:, dj * 128:(dj + 1) * 128],
                                        ident[:])
                    nc.vector.tensor_copy(mT[:, dj, tb * 128:(tb + 1) * 128],
                                          ps[:])

        # ---------- Phase E: MLP (fp8 DoubleRow) ----------
        gt_pool = tc.tile_pool(name="gtpl", bufs=1)
        gtpl = gt_pool.__enter__()
        GT = gtpl.tile([128, 64, 512], mlp_dt, name="GT")
        wfcT_v = wfc_gath
        wpjT_v = wpj_gath
        MKS = 2 if F8_MLP else 1

        with tc.tile_pool(name="wfc", bufs=6) as wfcp, \
             tc.tile_pool(name="ups", bufs=2, space="PSUM") as ups:
            for grp in range(16):        # 4 f-tiles (512 features) per group
                ps = [ups.tile([128, 512], dt.float32, tag=f"u{i}",
                               name=f"u_{grp}_{i}") for i in range(4)]
                for djp in range(16 // MKS):
                    w2 = wfcp.tile([128, MKS, 512], mlp_dt, tag="wfct")
                    nc.sync.dma_start(
                        out=w2[:],
                        in_=bass.AP(tensor=wfcT_v.tensor,
                                    offset=wfcT_v.offset
                                    + djp * MKS * 128 * FF + grp * 512,
                                    ap=[[FF, 128], [128 * FF, MKS],
                                        [1, 512]]))
                    for f4 in range(4):
                        if F8_MLP:
                            nc.tensor.matmul(
                                ps[f4][:],
                                w2[:, :, f4 * 128:(f4 + 1) * 128],
                                mT[:, djp * 2:djp * 2 + 2, :],
                                start=(djp == 0), stop=(djp == 7),
                                perf_mode=DR)
                        else:
                            nc.tensor.matmul(
                                ps[f4][:],
                                w2[:, 0, f4 * 128:(f4 + 1) * 128],
                                mT[:, djp, :],
                                start=(djp == 0), stop=(djp == 15))
                for f4 in range(4):
                    fti = grp * 4 + f4
                    nc.scalar.activation(GT[:, fti, :], ps[f4][:],
                                         AF.Gelu_apprx_tanh,
                                         bias=bfc_pp[:, fti:fti + 1],
                                         scale=(1.0 / WS if F8_MLP else 1.0))

        with tc.tile_pool(name="wpj", bufs=4) as wpjp, \
             tc.tile_pool(name="yps", bufs=1, space="PSUM") as yps, \
             tc.tile_pool(name="outp", bufs=4) as outp:
            for ttp in range(2):
                ps = [yps.tile([128, 512], dt.float32, tag=f"y{i}",
                               name=f"y_{ttp}_{i}") for i in range(8)]
                for fp in range(64 // MKS):
                    wp2 = wpjp.tile([128, MKS, 2048], mlp_dt, tag="wpjt")
                    nc.sync.dma_start(
                        out=wp2[:],
                        in_=bass.AP(tensor=wpjT_v.tensor,
                                    offset=wpjT_v.offset
                                    + fp * MKS * 128 * D,
                                    ap=[[D, 128], [128 * D, MKS],
                                        [1, 2048]]))
                    for tt2 in range(2):
                        tt = ttp * 2 + tt2
                        for dg in range(4):
                            if F8_MLP:
                                nc.tensor.matmul(
                                    ps[tt2 * 4 + dg][:],
                                    GT[:, fp * 2:fp * 2 + 2,
                                       tt * 128:(tt + 1) * 128],
                                    wp2[:, :, dg * 512:(dg + 1) * 512],
                                    start=(fp == 0), stop=(fp == 31),
                                    perf_mode=DR)
                            else:
                                nc.tensor.matmul(
                                    ps[tt2 * 4 + dg][:],
                                    GT[:, fp, tt * 128:(tt + 1) * 128],
                                    wp2[:, 0, dg * 512:(dg + 1) * 512],
                                    start=(fp == 0), stop=(fp == 63))
                for tt2 in range(2):
                    tt = ttp * 2 + tt2
                    for dg in range(4):
                        sl = slice(dg * 512, dg * 512 + 512)
                        o_t = outp.tile([128, 512], dt.float32, tag="o")
                        if F8_MLP:
                            nc.vector.tensor_scalar(
                                o_t[:], ps[tt2 * 4 + dg][:], 1.0 / WS, None,
                                op0=OP.mult)
                            nc.vector.tensor_add(o_t[:], o_t[:],
                                                 bpj_bc[:, sl])
                        else:
                            nc.vector.tensor_add(o_t[:], ps[tt2 * 4 + dg][:],
                                                 bpj_bc[:, sl])
                        nc.vector.tensor_add(o_t[:], o_t[:],
                                             h2acc[tt][:, sl])
                        nc.sync.dma_start(
                            out=out_d[tt * 128:(tt + 1) * 128, sl],
                            in_=o_t[:])
        gt_pool.__exit__(None, None, None)
        h2_pool.__exit__(None, None, None)
        stack.close()

    nc.compile()
    return nc


def _host_prep(inputs):
    f32 = lambda k: np.ascontiguousarray(np.asarray(inputs[k], np.float32))
    x = f32("hidden_states")
    wqT = np.ascontiguousarray(f32("wq").T)
    wkT = np.ascontiguousarray(f32("wk").T)
    wvT = np.ascontiguousarray(f32("wv").T)
    woT = np.ascontiguousarray(f32("wo").T)
    wfcT = np.ascontiguousarray(f32("w_fc").T).ravel()
    wpjT = np.ascontiguousarray(f32("w_proj").T).ravel()
    # causal masks per core: q token = 8*qf + c, k token = 8*(ks*128+kp) + r
    # packed [128, 8, 384]: cols 0:256 = ksub0 x q[0:256],
    #                       cols 256:384 = ksub1 x q[128:256]
    kp = np.arange(128)[:, None, None]
    rr = np.arange(8)[None, :, None]
    ks = np.concatenate([np.zeros(256, np.int64),
                         np.ones(128, np.int64)])[None, None, :]
    qf = np.concatenate([np.arange(256),
                         np.arange(128, 256)])[None, None, :]
    in_maps = []
    for c in range(NC):
        mask = np.where(8 * (ks * 128 + kp) + rr <= 8 * qf + c,
                        0.0, -1e9).astype(np.float32)
        mask = mask.astype(ml_dtypes.bfloat16)
        in_maps.append({
            "xl": np.concatenate([x[0, c::NC, :], x[1, c::NC, :]], 0),
            "wqT": wqT, "wkT": wkT, "wvT": wvT, "woT": woT,
            "wfc_ch": wfcT[c * WFC_CH:(c + 1) * WFC_CH],
            "wpj_ch": wpjT[c * WPJ_CH:(c + 1) * WPJ_CH],
            "mask": mask,
            "ln1g": f32("ln1_g"), "ln1b": f32("ln1_b"),
            "ln2g": f32("ln2_g"), "ln2b": f32("ln2_b"),
            "bo": f32("bo"), "bfc": f32("b_fc"), "bpj": f32("b_proj"),
        })
    return in_maps


def kernel(**inputs) -> np.ndarray:
    in_maps = _host_prep(inputs)
    key = (not bool(np.all(np.asarray(inputs["ln1_g"]) == 1.0)),
           not bool(np.all(np.asarray(inputs["ln1_b"]) == 0.0)),
           not bool(np.all(np.asarray(inputs["ln2_g"]) == 1.0)),
           not bool(np.all(np.asarray(inputs["ln2_b"]) == 0.0)))
    if key not in _CACHE:
        _CACHE[key] = _build(*key)
    nc = _CACHE[key]
    res = run_bass_kernel_spmd(nc, in_maps, core_ids=list(range(NC)))
    if res.exec_time_ns is not None:
        print(f"HW exec time: {res.exec_time_ns} ns")
    out = np.zeros((B, S, D), np.float32)
    for c in range(NC):
        o = res.results[c]["out"]
        out[0, c::NC] = o[:CH]
        out[1, c::NC] = o[CH:]
    return out
